# revision 28
# baseline (speedup 1.0000x reference)
"""Embedding-lookup kernel for Trainium2 (Bass/Tile), 8-core data-parallel.

Problem: out[b, l] = prototypes[labels[b, l]]
  inputs     (512, 21, 1, 29, 129) f32  -- unused except for batch size
  labels     (512, 21) int64            -- values in [0, 25)
  prototypes (25, 1, 29, 129) f32
  out        (512, 21, 1, 29, 129) f32  (~161 MB)

Strategy (memory regime): shard the batch dim across 8 cores (64 batches =
1344 lookups per core). Per core, keep the tiny prototype table resident in
SBUF, build a one-hot matrix from the labels on device (PE ones-matmul
broadcast + iota/is_equal), and perform the gather as one-hot @ table
matmuls on the PE, streaming PSUM -> SBUF -> DRAM. HBM traffic is then
write-only (20.1 MB per core), which is the roofline for this problem.

f32 exactness: the PE's fp32 matmul is a 2-pass approximation, so the table
is split into three bf16 planes (hi/mid/lo) whose sum reconstructs every f32
exactly. The planes are stacked along the contraction dim (partition groups
0/32/64, K=96) so a single bf16 matmul accumulates hi+mid+lo in fp32 PSUM;
with 0/1 one-hot weights the gathered values are bit-exact.

Measured on 8 axon trn2 cores: ~67-80 us HW exec (bit-exact), vs a ~56 us
per-core HBM write floor + ~16 us fixed framework preamble/teardown.
"""

import json

import numpy as np

import concourse.bass as bass
import concourse.mybir as mybir
from concourse.tile import TileContext
from concourse.bass_utils import run_bass_kernel_spmd

B, L, NCHAN, T, F = 512, 21, 1, 29, 129
D = NCHAN * T * F            # 3741 features per prototype
N_PROTO = 25
N_CORES = 8
B_PER_CORE = B // N_CORES    # 64
ROWS = B_PER_CORE * L        # 1344 lookups per core

ROW_TILE = 128               # output rows per matmul (PSUM partition dim)
COL_TILE = 512               # output cols per matmul (one PSUM bank of f32)

# "v2" (exact; host-split bf16 planes, one matmul per tile), "k75" (exact,
# fully on-device split), "bf16x3" (exact, three matmuls per tile),
# "f32"/"f32r" (native fp32 PE paths, speed/exactness probes only),
# "v3" (~1e-5 rel err; K=64 two-plane split, latency-optimized pipeline).
_MODE = "v2"

GP = 32                  # partition stride between the three plane groups
KDIM = 3 * GP            # 96 = matmul contraction dim incl. zero pads


def _split_multiwaits(bir: dict) -> int:
    """This walrus build allows at most one sync-wait per instruction on
    several instruction encodings; Tile attaches one wait per dependency.
    Hoist every wait of a multi-wait instruction into its own EventSemaphore
    (the encoding `wait_ge` uses) inserted directly before it on the same
    engine. Returns the number of instructions split."""
    n_split = 0
    ctr = 0
    for f in bir["functions"]:
        for blk in f["blocks"]:
            insts = blk["instructions"]
            out = []
            for inst in insts:
                si = inst.get("sync_info")
                waits = (si or {}).get("on_wait") or []
                if len(waits) > 1:
                    n_split += 1
                    for w in waits:
                        ctr += 1
                        out.append(
                            {
                                "debug": inst.get("debug", 0),
                                "engine": inst["engine"],
                                "ins": [],
                                "outs": [],
                                "name": f"mwsplit-{ctr}",
                                "opcode": "EventSemaphore",
                                "sync_info": {"on_update": [], "on_wait": [w]},
                            }
                        )
                    si["on_wait"] = []
                out.append(inst)
            blk["instructions"] = out
    return n_split


def _install_multiwait_splitter(nc: bass.Bass) -> None:
    orig = nc.to_json_bytes

    def patched() -> bytes:
        bir = json.loads(orig())
        _split_multiwaits(bir)
        return json.dumps(bir).encode()

    nc.to_json_bytes = patched


def host_split_planes(proto: np.ndarray) -> np.ndarray:
    """Split the f32 table into hi/mid/lo bf16 planes (sum reconstructs every
    f32 exactly) laid out at partitions 0/32/64 with zero pads."""
    import ml_dtypes

    bf = ml_dtypes.bfloat16
    x = proto.astype(np.float32).reshape(N_PROTO, D)
    hi = x.astype(bf)
    r1 = x - hi.astype(np.float32)
    mid = r1.astype(bf)
    r2 = r1 - mid.astype(np.float32)
    lo = r2.astype(bf)
    planes = np.zeros((KDIM, D), dtype=bf)
    planes[0:N_PROTO] = hi
    planes[GP : GP + N_PROTO] = mid
    planes[2 * GP : 2 * GP + N_PROTO] = lo
    return planes


def host_split_planes2(proto: np.ndarray) -> np.ndarray:
    """Two bf16 planes (hi/mid) at partitions 0/32; sum reconstructs f32 to
    ~2^-17 relative error (harness gate is 2e-2)."""
    import ml_dtypes

    bf = ml_dtypes.bfloat16
    x = proto.astype(np.float32).reshape(N_PROTO, D)
    hi = x.astype(bf)
    mid = (x - hi.astype(np.float32)).astype(bf)
    planes = np.zeros((2 * GP, D), dtype=bf)
    planes[0:N_PROTO] = hi
    planes[GP : GP + N_PROTO] = mid
    return planes


def build_nc_v3() -> bass.Bass:
    """Latency-optimized gather: K=64 (hi/mid bf16 planes at partitions 0/32),
    labels broadcast to 64 partitions by DMA (PE starts directly on gather
    matmuls), planes loaded in two chunks on the Activation HWDGE queue,
    first output tile written per column-pair, output DMAs alternating
    between the Sync and Activation queues."""
    f32 = mybir.dt.float32
    bf16 = mybir.dt.bfloat16
    i32 = mybir.dt.int32
    K2 = 2 * GP              # 64

    nc = bass.Bass()
    lbl = nc.dram_tensor("lbl", [1, ROWS], f32, kind="ExternalInput")
    planes_in = nc.dram_tensor("planes", [K2, D], bf16, kind="ExternalInput")
    out = nc.dram_tensor("out", [ROWS, D], f32, kind="ExternalOutput")

    n_row_tiles = (ROWS + ROW_TILE - 1) // ROW_TILE   # 11
    OH_CHUNK = 448
    PAIR = 2 * COL_TILE                               # 1024
    n_pairs = (D + PAIR - 1) // PAIR                  # 4

    with TileContext(nc) as tc:
        with (
            tc.tile_pool(name="const", bufs=1) as cpool,
            tc.tile_pool(name="psum", bufs=4, space="PSUM") as ppool,
            tc.tile_pool(name="outp", bufs=8) as opool,
        ):
            # --- inputs: label broadcast on Sync queue, planes on Act queue
            lbl75 = cpool.tile([K2, ROWS], f32)
            nc.sync.dma_start(
                out=lbl75, in_=lbl[0].partition_broadcast(K2)
            )
            planes = cpool.tile([K2, D], bf16)
            nc.scalar.dma_start(out=planes[:, 0:PAIR], in_=planes_in[:, 0:PAIR])
            nc.scalar.dma_start(out=planes[:, PAIR:], in_=planes_in[:, PAIR:])

            # --- per-partition compare value: p & 31, clamped to 25 on pads
            iota_i = cpool.tile([K2, 1], i32)
            nc.gpsimd.iota(iota_i, pattern=[[0, 1]], base=0, channel_multiplier=1)
            iota_q = cpool.tile([K2, 1], i32)
            nc.vector.tensor_scalar(
                out=iota_q, in0=iota_i, scalar1=GP - 1, scalar2=None,
                op0=mybir.AluOpType.bitwise_and,
            )
            iota_m = cpool.tile([K2, 1], i32)
            nc.vector.tensor_scalar(
                out=iota_m, in0=iota_q, scalar1=N_PROTO, scalar2=None,
                op0=mybir.AluOpType.min,
            )
            iota_f = cpool.tile([K2, 1], f32)
            nc.vector.tensor_copy(out=iota_f, in_=iota_m)

            oh = cpool.tile([K2, ROWS], bf16)

            def build_oh_chunk(ch: int) -> None:
                c0 = ch * OH_CHUNK
                cw = min(OH_CHUNK, ROWS - c0)
                nc.vector.tensor_scalar(
                    out=oh[:, c0 : c0 + cw],
                    in0=lbl75[:, c0 : c0 + cw],
                    scalar1=iota_f[:, 0:1],
                    scalar2=None,
                    op0=mybir.AluOpType.is_equal,
                )

            build_oh_chunk(0)

            for r in range(n_row_tiles):
                pr = min(ROW_TILE, ROWS - r * ROW_TILE)
                ot = opool.tile([ROW_TILE, D], f32)
                oh_sl = oh[:, r * ROW_TILE : r * ROW_TILE + pr]
                for p in range(n_pairs):
                    c0 = p * PAIR
                    cw = min(PAIR, D - c0)
                    ps = ppool.tile([ROW_TILE, PAIR], f32)
                    for h in range(2):
                        hw = min(COL_TILE, cw - h * COL_TILE)
                        if hw <= 0:
                            break
                        nc.tensor.matmul(
                            ps[:pr, h * COL_TILE : h * COL_TILE + hw],
                            oh_sl,
                            planes[:, c0 + h * COL_TILE : c0 + h * COL_TILE + hw],
                            start=True,
                            stop=True,
                        )
                    dst = ot[:pr, c0 : c0 + cw]
                    if p % 2 == 0:
                        nc.vector.tensor_copy(out=dst, in_=ps[:pr, :cw])
                    else:
                        nc.scalar.copy(out=dst, in_=ps[:pr, :cw])
                    if r == 0:
                        # stream tile 0 per pair, alternating queues
                        eng = nc.sync if p % 2 == 0 else nc.scalar
                        eng.dma_start(
                            out=out[0:pr, c0 : c0 + cw],
                            in_=ot[:pr, c0 : c0 + cw],
                        )
                    elif r == 1 and p == 1:
                        nc.sync.dma_start(
                            out=out[ROW_TILE : ROW_TILE + pr, 0 : 2 * PAIR],
                            in_=ot[:pr, 0 : 2 * PAIR],
                        )
                if r == 1:
                    nc.scalar.dma_start(
                        out=out[ROW_TILE : ROW_TILE + pr, 2 * PAIR :],
                        in_=ot[:pr, 2 * PAIR :],
                    )
                elif r > 1:
                    eng = nc.sync if r % 2 == 0 else nc.scalar
                    eng.dma_start(
                        out=out[r * ROW_TILE : r * ROW_TILE + pr, :],
                        in_=ot[:pr, :],
                    )
                if r == 0:
                    build_oh_chunk(1)
                elif r == 1:
                    build_oh_chunk(2)
    _install_multiwait_splitter(nc)
    return nc


def build_nc_v4() -> bass.Bass:
    """v2's exact K=96 gather with a latency-optimized pipeline:
    - planes staged as two contiguous DRAM tensors, loaded on the Act queue
      (fully contiguous packets at full engine rate, off the Sync queue),
    - one-hot built per 448-row chunk, interleaved with the first tiles,
    - tile 0 streamed out per column-pair, tile 1 in halves,
    - output DMAs alternate between the Sync and Act HWDGE queues."""
    f32 = mybir.dt.float32
    bf16 = mybir.dt.bfloat16
    i32 = mybir.dt.int32
    PAIR = 2 * COL_TILE                               # 1024
    DA = 1024                                         # planesA cols
    n_pairs = (D + PAIR - 1) // PAIR                  # 4

    nc = bass.Bass()
    lbl = nc.dram_tensor("lbl", [1, ROWS], bf16, kind="ExternalInput")
    planesA_in = nc.dram_tensor("planesA", [KDIM, DA], bf16, kind="ExternalInput")
    planesB_in = nc.dram_tensor(
        "planesB", [KDIM, D - DA], bf16, kind="ExternalInput"
    )
    out = nc.dram_tensor("out", [ROWS, D], f32, kind="ExternalOutput")

    n_row_tiles = (ROWS + ROW_TILE - 1) // ROW_TILE   # 11
    OH_CHUNK = 448

    with TileContext(nc) as tc:
        with (
            tc.tile_pool(name="const", bufs=1) as cpool,
            tc.tile_pool(name="psum", bufs=4, space="PSUM") as ppool,
            tc.tile_pool(name="outp", bufs=8) as opool,
        ):
            lblsb = cpool.tile([1, ROWS], bf16)
            nc.sync.dma_start(out=lblsb, in_=lbl[:])

            planes = cpool.tile([KDIM, D], bf16)
            nc.scalar.dma_start(out=planes[:, 0:DA], in_=planesA_in[:])
            nc.scalar.dma_start(out=planes[:, DA:], in_=planesB_in[:])

            ones = cpool.tile([1, KDIM], bf16)
            nc.vector.memset(ones, 1.0)
            iota_i = cpool.tile([KDIM, 1], i32)
            nc.gpsimd.iota(iota_i, pattern=[[0, 1]], base=0, channel_multiplier=1)
            iota_q = cpool.tile([KDIM, 1], i32)
            nc.vector.tensor_scalar(
                out=iota_q, in0=iota_i, scalar1=GP - 1, scalar2=None,
                op0=mybir.AluOpType.bitwise_and,
            )
            iota_m = cpool.tile([KDIM, 1], i32)
            nc.vector.tensor_scalar(
                out=iota_m, in0=iota_q, scalar1=N_PROTO, scalar2=None,
                op0=mybir.AluOpType.min,
            )
            iota_f = cpool.tile([KDIM, 1], f32)
            nc.vector.tensor_copy(out=iota_f, in_=iota_m)

            oh = cpool.tile([KDIM, ROWS], bf16)

            def build_oh_chunk(ch: int) -> None:
                c0 = ch * OH_CHUNK
                cw = min(OH_CHUNK, ROWS - c0)
                pb = ppool.tile([ROW_TILE, PAIR], f32, tag="ps")
                nc.tensor.matmul(
                    pb[:KDIM, :cw],
                    ones[0:1, :],
                    lblsb[0:1, c0 : c0 + cw],
                    start=True,
                    stop=True,
                )
                nc.vector.tensor_scalar(
                    out=oh[:, c0 : c0 + cw],
                    in0=pb[:KDIM, :cw],
                    scalar1=iota_f[:, 0:1],
                    scalar2=None,
                    op0=mybir.AluOpType.is_equal,
                )

            build_oh_chunk(0)

            for r in range(n_row_tiles):
                pr = min(ROW_TILE, ROWS - r * ROW_TILE)
                ot = opool.tile([ROW_TILE, D], f32)
                oh_sl = oh[:, r * ROW_TILE : r * ROW_TILE + pr]
                for p in range(n_pairs):
                    c0 = p * PAIR
                    cw = min(PAIR, D - c0)
                    ps = ppool.tile([ROW_TILE, PAIR], f32, tag="ps")
                    for h in range(2):
                        hw = min(COL_TILE, cw - h * COL_TILE)
                        if hw <= 0:
                            break
                        nc.tensor.matmul(
                            ps[:pr, h * COL_TILE : h * COL_TILE + hw],
                            oh_sl,
                            planes[:, c0 + h * COL_TILE : c0 + h * COL_TILE + hw],
                            start=True,
                            stop=True,
                        )
                    dst = ot[:pr, c0 : c0 + cw]
                    if p % 2 == 0:
                        nc.vector.tensor_copy(out=dst, in_=ps[:pr, :cw])
                    else:
                        nc.scalar.copy(out=dst, in_=ps[:pr, :cw])
                    if r == 0:
                        eng = nc.sync if p % 2 == 0 else nc.scalar
                        eng.dma_start(
                            out=out[0:pr, c0 : c0 + cw],
                            in_=ot[:pr, c0 : c0 + cw],
                        )
                    elif r == 1 and p == 1:
                        nc.sync.dma_start(
                            out=out[ROW_TILE : ROW_TILE + pr, 0 : 2 * PAIR],
                            in_=ot[:pr, 0 : 2 * PAIR],
                        )
                if r == 1:
                    nc.scalar.dma_start(
                        out=out[ROW_TILE : ROW_TILE + pr, 2 * PAIR :],
                        in_=ot[:pr, 2 * PAIR :],
                    )
                elif r > 1:
                    eng = nc.sync if r % 2 == 0 else nc.scalar
                    eng.dma_start(
                        out=out[r * ROW_TILE : r * ROW_TILE + pr, :],
                        in_=ot[:pr, :],
                    )
                if r == 0:
                    build_oh_chunk(1)
                elif r == 1:
                    build_oh_chunk(2)
    _install_multiwait_splitter(nc)
    return nc


def build_nc_v5() -> bass.Bass:
    """Grant-aware overlap: epoch-1 HBM stays light (sync-queue only, tile-0
    pair DMAs ~1.9MB) so the power governor grants full PE clock at ~20us;
    after that the stream goes wide: per-half-tile DMAs split between the
    Sync and Act HWDGE queues, so each DMA waits on only two PSUM copies."""
    f32 = mybir.dt.float32
    bf16 = mybir.dt.bfloat16
    i32 = mybir.dt.int32
    PAIR = 2 * COL_TILE                               # 1024
    DA = 1024                                         # planesA cols
    n_pairs = (D + PAIR - 1) // PAIR                  # 4
    HALF = 2 * PAIR                                   # 2048 cols per DMA half

    nc = bass.Bass()
    lbl = nc.dram_tensor("lbl", [1, ROWS], bf16, kind="ExternalInput")
    planesA_in = nc.dram_tensor("planesA", [KDIM, DA], bf16, kind="ExternalInput")
    planesB_in = nc.dram_tensor(
        "planesB", [KDIM, D - DA], bf16, kind="ExternalInput"
    )
    out = nc.dram_tensor("out", [ROWS, D], f32, kind="ExternalOutput")

    n_row_tiles = (ROWS + ROW_TILE - 1) // ROW_TILE   # 11
    OH_CHUNK = 448

    with TileContext(nc) as tc:
        with (
            tc.tile_pool(name="const", bufs=1) as cpool,
            tc.tile_pool(name="psum", bufs=4, space="PSUM") as ppool,
            tc.tile_pool(name="outp", bufs=8) as opool,
        ):
            lblsb = cpool.tile([1, ROWS], bf16)
            nc.sync.dma_start(out=lblsb, in_=lbl[:])

            planes = cpool.tile([KDIM, D], bf16)
            nc.sync.dma_start(out=planes[:, 0:DA], in_=planesA_in[:])
            nc.sync.dma_start(out=planes[:, DA:], in_=planesB_in[:])

            ones = cpool.tile([1, KDIM], bf16)
            nc.vector.memset(ones, 1.0)
            iota_i = cpool.tile([KDIM, 1], i32)
            nc.gpsimd.iota(iota_i, pattern=[[0, 1]], base=0, channel_multiplier=1)
            iota_q = cpool.tile([KDIM, 1], i32)
            nc.vector.tensor_scalar(
                out=iota_q, in0=iota_i, scalar1=GP - 1, scalar2=None,
                op0=mybir.AluOpType.bitwise_and,
            )
            iota_m = cpool.tile([KDIM, 1], i32)
            nc.vector.tensor_scalar(
                out=iota_m, in0=iota_q, scalar1=N_PROTO, scalar2=None,
                op0=mybir.AluOpType.min,
            )
            iota_f = cpool.tile([KDIM, 1], f32)
            nc.vector.tensor_copy(out=iota_f, in_=iota_m)

            oh = cpool.tile([KDIM, ROWS], bf16)

            def build_oh_chunk(ch: int) -> None:
                c0 = ch * OH_CHUNK
                cw = min(OH_CHUNK, ROWS - c0)
                pb = ppool.tile([ROW_TILE, PAIR], f32, tag="ps")
                nc.tensor.matmul(
                    pb[:KDIM, :cw],
                    ones[0:1, :],
                    lblsb[0:1, c0 : c0 + cw],
                    start=True,
                    stop=True,
                )
                nc.vector.tensor_scalar(
                    out=oh[:, c0 : c0 + cw],
                    in0=pb[:KDIM, :cw],
                    scalar1=iota_f[:, 0:1],
                    scalar2=None,
                    op0=mybir.AluOpType.is_equal,
                )

            build_oh_chunk(0)

            for r in range(n_row_tiles):
                pr = min(ROW_TILE, ROWS - r * ROW_TILE)
                r0 = r * ROW_TILE
                ot = opool.tile([ROW_TILE, D], f32)
                oh_sl = oh[:, r0 : r0 + pr]
                for p in range(n_pairs):
                    c0 = p * PAIR
                    cw = min(PAIR, D - c0)
                    ps = ppool.tile([ROW_TILE, PAIR], f32, tag="ps")
                    for h in range(2):
                        hw = min(COL_TILE, cw - h * COL_TILE)
                        if hw <= 0:
                            break
                        nc.tensor.matmul(
                            ps[:pr, h * COL_TILE : h * COL_TILE + hw],
                            oh_sl,
                            planes[:, c0 + h * COL_TILE : c0 + h * COL_TILE + hw],
                            start=True,
                            stop=True,
                        )
                    dst = ot[:pr, c0 : c0 + cw]
                    if p % 2 == 0:
                        nc.vector.tensor_copy(out=dst, in_=ps[:pr, :cw])
                    else:
                        nc.scalar.copy(out=dst, in_=ps[:pr, :cw])
                    if r == 0:
                        # epoch-1: keep traffic on the sync queue only
                        nc.sync.dma_start(
                            out=out[0:pr, c0 : c0 + cw],
                            in_=ot[:pr, c0 : c0 + cw],
                        )
                    elif p == 1:
                        # first half (cols 0:2048) after its two copies
                        nc.sync.dma_start(
                            out=out[r0 : r0 + pr, 0:HALF], in_=ot[:pr, 0:HALF]
                        )
                if r > 0:
                    nc.scalar.dma_start(
                        out=out[r0 : r0 + pr, HALF:], in_=ot[:pr, HALF:]
                    )
                if r == 0:
                    build_oh_chunk(1)
                elif r == 1:
                    build_oh_chunk(2)
    _install_multiwait_splitter(nc)
    return nc


K4 = 128                     # v6 contraction dim (5 packed fp8 plane groups)
N_PLANES = 5                 # planes packed at stride N_PROTO (25)
F8_SHIFT = 4                 # per-plane residual scale 2^4 (e4m3 mantissa)


def host_split_planes_f8(proto: np.ndarray) -> np.ndarray:
    """Five fp8-e4m3 planes packed at partition stride 25 (partitions
    0-124). Residuals are scaled by 2^4 per level (the matching one-hot
    carries 2^-4g in bf16), reconstructing f32 to ~2^-20 relative error with
    an absolute floor of 2^-26 — safe for the ~2^-16 minimum prototype
    magnitude. fp8 moving data runs the PE at 2x bf16 throughput."""
    import ml_dtypes

    f8 = ml_dtypes.float8_e4m3
    s = float(2 ** F8_SHIFT)
    x = proto.astype(np.float32).reshape(N_PROTO, D)
    planes = np.zeros((K4, D), dtype=f8)
    r = x
    for g in range(N_PLANES):
        p = r.astype(f8)
        planes[g * N_PROTO : (g + 1) * N_PROTO] = p
        r = (r - p.astype(np.float32)) * s
    return planes


def host_cmpscl() -> np.ndarray:
    """[128, 2] f32: col 0 = per-partition compare value (p mod 25, sentinel
    25 on pad partitions), col 1 = one-hot magnitude 2^(-4 * group)."""
    p = np.arange(K4)
    g = p // N_PROTO
    cmp = np.where(p < N_PLANES * N_PROTO, p % N_PROTO, N_PROTO).astype(np.float32)
    scl = np.where(
        p < N_PLANES * N_PROTO, 2.0 ** (-F8_SHIFT * g), 0.0
    ).astype(np.float32)
    return np.stack([cmp, scl], axis=1)


def build_nc_v6() -> bass.Bass:
    """fp8 gather with v2's proven DMA pattern: all output DMAs on the Sync
    queue, tile 0 primed per column pair, tiles 1+ with one DMA per tile and
    two PSUM->SBUF copies ([128,2048] DVE + [128,1693] ACT). The fp8 moving
    operand keeps the PE producer ahead of the drain regardless of the
    clock-governor state, so the stream is DMA-limited end to end."""
    f32 = mybir.dt.float32
    bf16 = mybir.dt.bfloat16
    f8 = mybir.dt.float8e4
    PAIR = 2 * COL_TILE                               # 1024
    HALF = 2 * PAIR                                   # 2048
    DA = 1024

    nc = bass.Bass()
    lbl = nc.dram_tensor("lbl", [1, ROWS], bf16, kind="ExternalInput")
    cs_in = nc.dram_tensor("cmpscl", [K4, 2], f32, kind="ExternalInput")
    planesA_in = nc.dram_tensor("planesA", [K4, DA], f8, kind="ExternalInput")
    planesB_in = nc.dram_tensor("planesB", [K4, D - DA], f8, kind="ExternalInput")
    out = nc.dram_tensor("out", [ROWS, D], f32, kind="ExternalOutput")

    n_row_tiles = (ROWS + ROW_TILE - 1) // ROW_TILE   # 11
    OH_CHUNK = 448

    with TileContext(nc) as tc:
        with (
            tc.tile_pool(name="const", bufs=1) as cpool,
            tc.tile_pool(name="psum", bufs=4, space="PSUM") as ppool,
            tc.tile_pool(name="outp", bufs=8) as opool,
        ):
            lblsb = cpool.tile([1, ROWS], bf16)
            nc.sync.dma_start(out=lblsb, in_=lbl[:])
            cs = cpool.tile([K4, 2], f32)
            nc.sync.dma_start(out=cs, in_=cs_in[:])
            planes = cpool.tile([K4, D], f8)
            nc.sync.dma_start(out=planes[:, 0:DA], in_=planesA_in[:])
            nc.sync.dma_start(out=planes[:, DA:], in_=planesB_in[:])

            ones = cpool.tile([1, K4], bf16)
            nc.vector.memset(ones, 1.0)

            oh = cpool.tile([K4, ROWS], bf16)

            def build_oh_chunk(ch: int) -> None:
                c0 = ch * OH_CHUNK
                cw = min(OH_CHUNK, ROWS - c0)
                pb = ppool.tile([ROW_TILE, PAIR], f32, tag="ps")
                nc.tensor.matmul(
                    pb[:K4, :cw],
                    ones[0:1, :],
                    lblsb[0:1, c0 : c0 + cw],
                    start=True,
                    stop=True,
                )
                nc.vector.tensor_scalar(
                    out=oh[:, c0 : c0 + cw],
                    in0=pb[:K4, :cw],
                    scalar1=cs[:, 0:1],
                    scalar2=cs[:, 1:2],
                    op0=mybir.AluOpType.is_equal,
                    op1=mybir.AluOpType.mult,
                )

            build_oh_chunk(0)

            for r in range(n_row_tiles):
                pr = min(ROW_TILE, ROWS - r * ROW_TILE)
                r0 = r * ROW_TILE
                ot = opool.tile([ROW_TILE, D], f32)
                oh_sl = oh[:, r0 : r0 + pr]
                for p in range(4):
                    c0 = p * PAIR
                    cw = min(PAIR, D - c0)
                    ps = ppool.tile([ROW_TILE, PAIR], f32, tag="ps")
                    for h in range(2):
                        hw = min(COL_TILE, cw - h * COL_TILE)
                        if hw <= 0:
                            break
                        nc.tensor.matmul(
                            ps[:pr, h * COL_TILE : h * COL_TILE + hw],
                            oh_sl,
                            planes[:, c0 + h * COL_TILE : c0 + h * COL_TILE + hw],
                            start=True,
                            stop=True,
                        )
                    dst = ot[:pr, c0 : c0 + cw]
                    if p % 2 == 0:
                        nc.vector.tensor_copy(out=dst, in_=ps[:pr, :cw])
                    else:
                        nc.scalar.copy(out=dst, in_=ps[:pr, :cw])
                    if r == 0:
                        # prime per column pair for the earliest first write
                        nc.sync.dma_start(
                            out=out[0:pr, c0 : c0 + cw],
                            in_=ot[:pr, c0 : c0 + cw],
                        )
                if r > 0:
                    nc.sync.dma_start(
                        out=out[r0 : r0 + pr, :], in_=ot[:pr, :]
                    )
                if r == 0:
                    build_oh_chunk(1)
                elif r == 1:
                    build_oh_chunk(2)
    _install_multiwait_splitter(nc)
    return nc


def build_nc_v7() -> bass.Bass:
    """v2 with two low-risk deltas: planes staged as two contiguous DRAM
    tensors (2 sync DMAs instead of 8 strided chunk loads -> sync engine free
    ~2.5us earlier, so the one-hot build and tile-0 primes start sooner) and
    a deeper output pool (10 bufs) so the producer never waits on slots."""
    f32 = mybir.dt.float32
    bf16 = mybir.dt.bfloat16
    i32 = mybir.dt.int32
    DA = 1024

    nc = bass.Bass()
    lbl = nc.dram_tensor("lbl", [1, ROWS], bf16, kind="ExternalInput")
    planesA_in = nc.dram_tensor("planesA", [KDIM, DA], bf16, kind="ExternalInput")
    planesB_in = nc.dram_tensor(
        "planesB", [KDIM, D - DA], bf16, kind="ExternalInput"
    )
    out = nc.dram_tensor("out", [ROWS, D], f32, kind="ExternalOutput")

    n_row_tiles = (ROWS + ROW_TILE - 1) // ROW_TILE
    n_col_tiles = (D + COL_TILE - 1) // COL_TILE
    OH_CHUNK = 448
    n_oh_chunks = (ROWS + OH_CHUNK - 1) // OH_CHUNK

    with TileContext(nc) as tc:
        with (
            tc.tile_pool(name="const", bufs=1) as cpool,
            tc.tile_pool(name="psum", bufs=4, space="PSUM") as ppool,
            tc.tile_pool(name="outp", bufs=10) as opool,
        ):
            lblsb = cpool.tile([1, ROWS], bf16)
            nc.sync.dma_start(out=lblsb, in_=lbl[:])

            planes = cpool.tile([KDIM, D], bf16)
            nc.sync.dma_start(out=planes[:, 0:DA], in_=planesA_in[:])
            nc.sync.dma_start(out=planes[:, DA:], in_=planesB_in[:])

            ones = cpool.tile([1, KDIM], bf16)
            nc.vector.memset(ones, 1.0)

            iota_i = cpool.tile([KDIM, 1], i32)
            nc.gpsimd.iota(iota_i, pattern=[[0, 1]], base=0, channel_multiplier=1)
            iota_q = cpool.tile([KDIM, 1], i32)
            nc.vector.tensor_scalar(
                out=iota_q, in0=iota_i, scalar1=GP - 1, scalar2=None,
                op0=mybir.AluOpType.bitwise_and,
            )
            iota_m = cpool.tile([KDIM, 1], i32)
            nc.vector.tensor_scalar(
                out=iota_m, in0=iota_q, scalar1=N_PROTO, scalar2=None,
                op0=mybir.AluOpType.min,
            )
            iota_f = cpool.tile([KDIM, 1], f32)
            nc.vector.tensor_copy(out=iota_f, in_=iota_m)

            oh = cpool.tile([KDIM, ROWS], bf16)
            for ch in range(n_oh_chunks):
                cw = min(OH_CHUNK, ROWS - ch * OH_CHUNK)
                pb = ppool.tile([ROW_TILE, COL_TILE], f32, tag="ps")
                nc.tensor.matmul(
                    pb[:KDIM, :cw],
                    ones[0:1, :],
                    lblsb[0:1, ch * OH_CHUNK : ch * OH_CHUNK + cw],
                    start=True,
                    stop=True,
                )
                nc.vector.tensor_scalar(
                    out=oh[:, ch * OH_CHUNK : ch * OH_CHUNK + cw],
                    in0=pb[:KDIM, :cw],
                    scalar1=iota_f[:, 0:1],
                    scalar2=None,
                    op0=mybir.AluOpType.is_equal,
                )

            n_pairs = (n_col_tiles + 1) // 2
            for r in range(n_row_tiles):
                pr = min(ROW_TILE, ROWS - r * ROW_TILE)
                ot = opool.tile([ROW_TILE, D], f32)
                oh_sl = oh[:, r * ROW_TILE : r * ROW_TILE + pr]
                for cp in range(n_pairs):
                    c0 = 2 * cp * COL_TILE
                    cw = min(2 * COL_TILE, D - c0)
                    ps = ppool.tile([ROW_TILE, 2 * COL_TILE], f32, tag="ps")
                    for h in range(2):
                        hw = min(COL_TILE, cw - h * COL_TILE)
                        if hw <= 0:
                            break
                        nc.tensor.matmul(
                            ps[:pr, h * COL_TILE : h * COL_TILE + hw],
                            oh_sl,
                            planes[:, c0 + h * COL_TILE : c0 + h * COL_TILE + hw],
                            start=True,
                            stop=True,
                        )
                    dst = ot[:pr, c0 : c0 + cw]
                    if cp % 2 == 1:
                        nc.scalar.copy(out=dst, in_=ps[:pr, :cw])
                    else:
                        nc.vector.tensor_copy(out=dst, in_=ps[:pr, :cw])
                    if r == 0 and cp in (0, 1):
                        nc.sync.dma_start(
                            out=out[0:pr, c0 : c0 + cw],
                            in_=ot[:pr, c0 : c0 + cw],
                        )
                if r == 0:
                    nc.sync.dma_start(
                        out=out[0:pr, 4 * COL_TILE :],
                        in_=ot[:pr, 4 * COL_TILE :],
                    )
                else:
                    nc.sync.dma_start(
                        out=out[r * ROW_TILE : r * ROW_TILE + pr, :], in_=ot[:pr, :]
                    )
    _install_multiwait_splitter(nc)
    return nc


def build_nc_v8() -> bass.Bass:
    """v7 with per-tile DMA row rotation: each tile's output DMA is split at
    a varying row offset so the row->DMA-engine assignment rotates tile to
    tile, spreading address-linked slow patches (HBM contention bursts that
    otherwise pile onto one engine) across all 16 engines."""
    f32 = mybir.dt.float32
    bf16 = mybir.dt.bfloat16
    i32 = mybir.dt.int32
    DA = 1024

    nc = bass.Bass()
    lbl = nc.dram_tensor("lbl", [1, ROWS], bf16, kind="ExternalInput")
    planesA_in = nc.dram_tensor("planesA", [KDIM, DA], bf16, kind="ExternalInput")
    planesB_in = nc.dram_tensor(
        "planesB", [KDIM, D - DA], bf16, kind="ExternalInput"
    )
    out = nc.dram_tensor("out", [ROWS, D], f32, kind="ExternalOutput")

    n_row_tiles = (ROWS + ROW_TILE - 1) // ROW_TILE
    n_col_tiles = (D + COL_TILE - 1) // COL_TILE
    OH_CHUNK = 448
    n_oh_chunks = (ROWS + OH_CHUNK - 1) // OH_CHUNK

    with TileContext(nc) as tc:
        with (
            tc.tile_pool(name="const", bufs=1) as cpool,
            tc.tile_pool(name="psum", bufs=4, space="PSUM") as ppool,
            tc.tile_pool(name="outp", bufs=8) as opool,
        ):
            lblsb = cpool.tile([1, ROWS], bf16)
            nc.sync.dma_start(out=lblsb, in_=lbl[:])

            planes = cpool.tile([KDIM, D], bf16)
            nc.sync.dma_start(out=planes[:, 0:DA], in_=planesA_in[:])
            nc.sync.dma_start(out=planes[:, DA:], in_=planesB_in[:])

            ones = cpool.tile([1, KDIM], bf16)
            nc.vector.memset(ones, 1.0)

            iota_i = cpool.tile([KDIM, 1], i32)
            nc.gpsimd.iota(iota_i, pattern=[[0, 1]], base=0, channel_multiplier=1)
            iota_q = cpool.tile([KDIM, 1], i32)
            nc.vector.tensor_scalar(
                out=iota_q, in0=iota_i, scalar1=GP - 1, scalar2=None,
                op0=mybir.AluOpType.bitwise_and,
            )
            iota_m = cpool.tile([KDIM, 1], i32)
            nc.vector.tensor_scalar(
                out=iota_m, in0=iota_q, scalar1=N_PROTO, scalar2=None,
                op0=mybir.AluOpType.min,
            )
            iota_f = cpool.tile([KDIM, 1], f32)
            nc.vector.tensor_copy(out=iota_f, in_=iota_m)

            oh = cpool.tile([KDIM, ROWS], bf16)
            for ch in range(n_oh_chunks):
                cw = min(OH_CHUNK, ROWS - ch * OH_CHUNK)
                pb = ppool.tile([ROW_TILE, COL_TILE], f32, tag="ps")
                nc.tensor.matmul(
                    pb[:KDIM, :cw],
                    ones[0:1, :],
                    lblsb[0:1, ch * OH_CHUNK : ch * OH_CHUNK + cw],
                    start=True,
                    stop=True,
                )
                nc.vector.tensor_scalar(
                    out=oh[:, ch * OH_CHUNK : ch * OH_CHUNK + cw],
                    in0=pb[:KDIM, :cw],
                    scalar1=iota_f[:, 0:1],
                    scalar2=None,
                    op0=mybir.AluOpType.is_equal,
                )

            n_pairs = (n_col_tiles + 1) // 2
            for r in range(n_row_tiles):
                pr = min(ROW_TILE, ROWS - r * ROW_TILE)
                r0 = r * ROW_TILE
                ot = opool.tile([ROW_TILE, D], f32)
                oh_sl = oh[:, r0 : r0 + pr]
                for cp in range(n_pairs):
                    c0 = 2 * cp * COL_TILE
                    cw = min(2 * COL_TILE, D - c0)
                    ps = ppool.tile([ROW_TILE, 2 * COL_TILE], f32, tag="ps")
                    for h in range(2):
                        hw = min(COL_TILE, cw - h * COL_TILE)
                        if hw <= 0:
                            break
                        nc.tensor.matmul(
                            ps[:pr, h * COL_TILE : h * COL_TILE + hw],
                            oh_sl,
                            planes[:, c0 + h * COL_TILE : c0 + h * COL_TILE + hw],
                            start=True,
                            stop=True,
                        )
                    dst = ot[:pr, c0 : c0 + cw]
                    if cp % 2 == 1:
                        nc.scalar.copy(out=dst, in_=ps[:pr, :cw])
                    else:
                        nc.vector.tensor_copy(out=dst, in_=ps[:pr, :cw])
                    if r == 0 and cp in (0, 1):
                        nc.sync.dma_start(
                            out=out[0:pr, c0 : c0 + cw],
                            in_=ot[:pr, c0 : c0 + cw],
                        )
                if r == 0:
                    nc.sync.dma_start(
                        out=out[0:pr, 4 * COL_TILE :],
                        in_=ot[:pr, 4 * COL_TILE :],
                    )
                else:
                    s = (5 * r) % 16
                    if s == 0 or pr < ROW_TILE:
                        nc.sync.dma_start(
                            out=out[r0 : r0 + pr, :], in_=ot[:pr, :]
                        )
                    else:
                        nc.sync.dma_start(
                            out=out[r0 + s : r0 + pr, :], in_=ot[s:pr, :]
                        )
                        nc.sync.dma_start(
                            out=out[r0 : r0 + s, :], in_=ot[:s, :]
                        )
    _install_multiwait_splitter(nc)
    return nc


def build_nc_v2() -> bass.Bass:
    """Gather as one-hot @ planes matmul, K=96 (three bf16 planes of the
    table stacked along the contraction dim, pre-split on host). One matmul
    per 128x512 output tile; PSUM->SBUF copies alternate DVE/ACT; one DMA
    per 128-row tile."""
    f32 = mybir.dt.float32
    bf16 = mybir.dt.bfloat16
    i32 = mybir.dt.int32

    nc = bass.Bass()
    lbl = nc.dram_tensor("lbl", [1, ROWS], bf16, kind="ExternalInput")
    planes_in = nc.dram_tensor("planes", [KDIM, D], bf16, kind="ExternalInput")
    out = nc.dram_tensor("out", [ROWS, D], f32, kind="ExternalOutput")

    n_row_tiles = (ROWS + ROW_TILE - 1) // ROW_TILE
    n_col_tiles = (D + COL_TILE - 1) // COL_TILE
    OH_CHUNK = 448
    n_oh_chunks = (ROWS + OH_CHUNK - 1) // OH_CHUNK

    with TileContext(nc) as tc:
        with (
            tc.tile_pool(name="const", bufs=1) as cpool,
            tc.tile_pool(name="psum", bufs=4, space="PSUM") as ppool,
            tc.tile_pool(name="outp", bufs=8) as opool,
        ):
            lblsb = cpool.tile([1, ROWS], bf16)
            nc.sync.dma_start(out=lblsb, in_=lbl[:])

            planes = cpool.tile([KDIM, D], bf16)
            for c in range(n_col_tiles):
                cn = min(COL_TILE, D - c * COL_TILE)
                nc.sync.dma_start(
                    out=planes[:, c * COL_TILE : c * COL_TILE + cn],
                    in_=planes_in[:, c * COL_TILE : c * COL_TILE + cn],
                )
            ones = cpool.tile([1, KDIM], bf16)
            nc.vector.memset(ones, 1.0)

            iota_i = cpool.tile([KDIM, 1], i32)
            nc.gpsimd.iota(iota_i, pattern=[[0, 1]], base=0, channel_multiplier=1)
            iota_q = cpool.tile([KDIM, 1], i32)
            nc.vector.tensor_scalar(
                out=iota_q, in0=iota_i, scalar1=GP - 1, scalar2=None,
                op0=mybir.AluOpType.bitwise_and,
            )
            iota_m = cpool.tile([KDIM, 1], i32)
            nc.vector.tensor_scalar(
                out=iota_m, in0=iota_q, scalar1=N_PROTO, scalar2=None,
                op0=mybir.AluOpType.min,
            )
            iota_f = cpool.tile([KDIM, 1], f32)
            nc.vector.tensor_copy(out=iota_f, in_=iota_m)

            # broadcast labels to 96 partitions on the (idle) PE: ones^T @ lbl,
            # then compare against the per-partition group-local iota
            oh = cpool.tile([KDIM, ROWS], bf16)
            for ch in range(n_oh_chunks):
                cw = min(OH_CHUNK, ROWS - ch * OH_CHUNK)
                pb = ppool.tile([ROW_TILE, COL_TILE], f32, tag="ps")
                nc.tensor.matmul(
                    pb[:KDIM, :cw],
                    ones[0:1, :],
                    lblsb[0:1, ch * OH_CHUNK : ch * OH_CHUNK + cw],
                    start=True,
                    stop=True,
                )
                nc.vector.tensor_scalar(
                    out=oh[:, ch * OH_CHUNK : ch * OH_CHUNK + cw],
                    in0=pb[:KDIM, :cw],
                    scalar1=iota_f[:, 0:1],
                    scalar2=None,
                    op0=mybir.AluOpType.is_equal,
                )

            n_pairs = (n_col_tiles + 1) // 2
            for r in range(n_row_tiles):
                pr = min(ROW_TILE, ROWS - r * ROW_TILE)
                ot = opool.tile([ROW_TILE, D], f32)
                oh_sl = oh[:, r * ROW_TILE : r * ROW_TILE + pr]
                for cp in range(n_pairs):
                    c0 = 2 * cp * COL_TILE
                    cw = min(2 * COL_TILE, D - c0)
                    ps = ppool.tile([ROW_TILE, 2 * COL_TILE], f32)
                    for h in range(2):
                        hw = min(COL_TILE, cw - h * COL_TILE)
                        if hw <= 0:
                            break
                        nc.tensor.matmul(
                            ps[:pr, h * COL_TILE : h * COL_TILE + hw],
                            oh_sl,
                            planes[:, c0 + h * COL_TILE : c0 + h * COL_TILE + hw],
                            start=True,
                            stop=True,
                        )
                    dst = ot[:pr, c0 : c0 + cw]
                    if cp % 2 == 1:
                        nc.scalar.copy(out=dst, in_=ps[:pr, :cw])
                    else:
                        nc.vector.tensor_copy(out=dst, in_=ps[:pr, :cw])
                    if r == 0 and cp in (0, 1):
                        # prime the output-DMA stream before the tile finishes
                        nc.sync.dma_start(
                            out=out[0:pr, c0 : c0 + cw],
                            in_=ot[:pr, c0 : c0 + cw],
                        )
                if r == 0:
                    nc.sync.dma_start(
                        out=out[0:pr, 4 * COL_TILE :],
                        in_=ot[:pr, 4 * COL_TILE :],
                    )
                else:
                    nc.sync.dma_start(
                        out=out[r * ROW_TILE : r * ROW_TILE + pr, :], in_=ot[:pr, :]
                    )
    _install_multiwait_splitter(nc)
    return nc


def build_nc_k75() -> bass.Bass:
    """One matmul per output tile: stationary is the 25-row one-hot stacked
    three times along the contraction dim, the moving operand is the
    hi/mid/lo bf16 table planes stacked the same way. PSUM accumulates
    hi+mid+lo in fp32 in a single pass -> bit-exact f32 gather.

    Compute-engine SBUF accesses must start at a 32-aligned partition, so the
    three 25-row groups sit at partitions 0/32/64 (K=96). Pad partitions:
    one-hot rows compare labels against 25 (never matches -> 0), plane pad
    rows are zeroed via DMA so 0*0 keeps PSUM clean."""
    f32 = mybir.dt.float32
    bf16 = mybir.dt.bfloat16
    i32 = mybir.dt.int32
    GP = 32                  # partition stride between plane groups
    P3 = 3 * GP              # 96 = contraction dim incl. pads

    nc = bass.Bass()
    lbl = nc.dram_tensor("lbl", [1, ROWS], f32, kind="ExternalInput")
    proto = nc.dram_tensor("proto", [N_PROTO, D], f32, kind="ExternalInput")
    out = nc.dram_tensor("out", [ROWS, D], f32, kind="ExternalOutput")

    n_row_tiles = (ROWS + ROW_TILE - 1) // ROW_TILE
    n_col_tiles = (D + COL_TILE - 1) // COL_TILE

    with TileContext(nc) as tc:
        with (
            tc.tile_pool(name="const", bufs=1) as cpool,
            tc.tile_pool(name="psum", bufs=8, space="PSUM") as ppool,
            tc.tile_pool(name="outp", bufs=4) as opool,
        ):
            tbl75 = cpool.tile([P3, D], f32)
            lbl75 = cpool.tile([P3, ROWS], f32)
            for g in range(3):
                sl = slice(g * GP, g * GP + N_PROTO)
                nc.sync.dma_start(out=tbl75[sl, :], in_=proto[:])
                nc.sync.dma_start(
                    out=lbl75[g * GP : (g + 1) * GP, :],
                    in_=lbl[0].partition_broadcast(GP),
                )

            iota_i = cpool.tile([P3, 1], i32)
            nc.gpsimd.iota(iota_i, pattern=[[0, 1]], base=0, channel_multiplier=1)
            # group-local index, pads clamp to 25 which no label ever equals
            iota_q = cpool.tile([P3, 1], i32)
            nc.vector.tensor_scalar(
                out=iota_q, in0=iota_i, scalar1=GP - 1, scalar2=None,
                op0=mybir.AluOpType.bitwise_and,
            )
            iota_m = cpool.tile([P3, 1], i32)
            nc.vector.tensor_scalar(
                out=iota_m, in0=iota_q, scalar1=N_PROTO, scalar2=None,
                op0=mybir.AluOpType.min,
            )
            iota_f = cpool.tile([P3, 1], f32)
            nc.vector.tensor_copy(out=iota_f, in_=iota_m)

            oh = cpool.tile([P3, ROWS], bf16)
            nc.vector.tensor_scalar(
                out=oh, in0=lbl75, scalar1=iota_f[:, 0:1], scalar2=None,
                op0=mybir.AluOpType.is_equal,
            )

            # planes: partitions 0-24 hi, 32-56 mid, 64-88 lo (bf16, RN)
            planes = cpool.tile([P3, D], bf16)
            scrA = cpool.tile([P3, D], f32)
            scrB = cpool.tile([P3, D], f32)
            zpad = cpool.tile([GP - N_PROTO, D], bf16)
            nc.vector.memset(zpad, 0.0)
            for g in range(3):
                nc.sync.dma_start(
                    out=planes[g * GP + N_PROTO : (g + 1) * GP, :], in_=zpad
                )
            s0 = slice(0, N_PROTO)
            s1 = slice(GP, GP + N_PROTO)
            s2 = slice(2 * GP, 2 * GP + N_PROTO)
            # hi plane
            nc.vector.tensor_copy(out=planes[s0, :], in_=tbl75[s0, :])
            # mid plane: cast(x - f32(bf16(x)))
            nc.vector.tensor_copy(out=planes[s1, :], in_=tbl75[s1, :])
            nc.vector.tensor_copy(out=scrA[s1, :], in_=planes[s1, :])
            nc.vector.tensor_sub(out=planes[s1, :], in0=tbl75[s1, :], in1=scrA[s1, :])
            # lo plane: r1 = x - hi_f; mid = bf16(r1); lo = bf16(r1 - f32(mid))
            nc.vector.tensor_copy(out=planes[s2, :], in_=tbl75[s2, :])
            nc.vector.tensor_copy(out=scrA[s2, :], in_=planes[s2, :])
            nc.vector.tensor_sub(out=scrB[s2, :], in0=tbl75[s2, :], in1=scrA[s2, :])
            nc.vector.tensor_copy(out=planes[s2, :], in_=scrB[s2, :])
            nc.vector.tensor_copy(out=scrA[s2, :], in_=planes[s2, :])
            nc.vector.tensor_sub(out=planes[s2, :], in0=scrB[s2, :], in1=scrA[s2, :])

            for r in range(n_row_tiles):
                pr = min(ROW_TILE, ROWS - r * ROW_TILE)
                ot = opool.tile([ROW_TILE, D], f32)
                oh_sl = oh[:, r * ROW_TILE : r * ROW_TILE + pr]
                for c in range(n_col_tiles):
                    cn = min(COL_TILE, D - c * COL_TILE)
                    ps = ppool.tile([ROW_TILE, COL_TILE], f32)
                    nc.tensor.matmul(
                        ps[:pr, :cn],
                        oh_sl,
                        planes[:, c * COL_TILE : c * COL_TILE + cn],
                        start=True,
                        stop=True,
                    )
                    dst = ot[:pr, c * COL_TILE : c * COL_TILE + cn]
                    if c in (3, 7):
                        nc.scalar.copy(out=dst, in_=ps[:pr, :cn])
                    else:
                        nc.vector.tensor_copy(out=dst, in_=ps[:pr, :cn])
                nc.sync.dma_start(
                    out=out[r * ROW_TILE : r * ROW_TILE + pr, :], in_=ot[:pr, :]
                )
    _install_multiwait_splitter(nc)
    return nc


def build_nc(mode: str = _MODE) -> bass.Bass:
    if mode == "v2":
        return build_nc_v2()
    if mode == "v3":
        return build_nc_v3()
    if mode == "v4":
        return build_nc_v4()
    if mode == "v5":
        return build_nc_v5()
    if mode == "v6":
        return build_nc_v6()
    if mode == "v7":
        return build_nc_v7()
    if mode == "v8":
        return build_nc_v8()
    if mode == "k75":
        return build_nc_k75()
    f32 = mybir.dt.float32
    bf16 = mybir.dt.bfloat16

    nc = bass.Bass()
    lbl = nc.dram_tensor("lbl", [1, ROWS], f32, kind="ExternalInput")
    proto = nc.dram_tensor("proto", [N_PROTO, D], f32, kind="ExternalInput")
    out = nc.dram_tensor("out", [ROWS, D], f32, kind="ExternalOutput")

    n_row_tiles = (ROWS + ROW_TILE - 1) // ROW_TILE
    n_col_tiles = (D + COL_TILE - 1) // COL_TILE

    with TileContext(nc) as tc:
        with (
            tc.tile_pool(name="const", bufs=1) as cpool,
            tc.tile_pool(name="psum", bufs=8, space="PSUM") as ppool,
            tc.tile_pool(name="outp", bufs=4) as opool,
        ):
            tbl = cpool.tile([N_PROTO, D], f32)
            nc.sync.dma_start(out=tbl, in_=proto[:])

            lblb = cpool.tile([N_PROTO, ROWS], f32)
            nc.sync.dma_start(out=lblb, in_=lbl[0].partition_broadcast(N_PROTO))

            iot = cpool.tile([N_PROTO, 1], f32)
            nc.gpsimd.iota(
                iot,
                pattern=[[0, 1]],
                base=0,
                channel_multiplier=1,
                allow_small_or_imprecise_dtypes=True,
            )

            oh_dt = f32 if mode in ("f32", "f32r") else bf16
            oh = cpool.tile([N_PROTO, ROWS], oh_dt)
            nc.vector.tensor_scalar(
                out=oh,
                in0=lblb,
                scalar1=iot[:, 0:1],
                scalar2=None,
                op0=mybir.AluOpType.is_equal,
            )

            if mode in ("f32", "f32r"):
                planes = [tbl]
            else:
                # Exact f32 = hi + mid + lo, each bf16 (RN cast at each step).
                hi = cpool.tile([N_PROTO, D], bf16)
                nc.vector.tensor_copy(out=hi, in_=tbl)
                hi_f = cpool.tile([N_PROTO, D], f32)
                nc.vector.tensor_copy(out=hi_f, in_=hi)
                r1 = cpool.tile([N_PROTO, D], f32)
                nc.vector.tensor_sub(out=r1, in0=tbl, in1=hi_f)
                mid = cpool.tile([N_PROTO, D], bf16)
                nc.vector.tensor_copy(out=mid, in_=r1)
                planes = [hi, mid]
                if mode == "bf16x3":
                    mid_f = cpool.tile([N_PROTO, D], f32)
                    nc.vector.tensor_copy(out=mid_f, in_=mid)
                    r2 = cpool.tile([N_PROTO, D], f32)
                    nc.vector.tensor_sub(out=r2, in0=r1, in1=mid_f)
                    lo = cpool.tile([N_PROTO, D], bf16)
                    nc.vector.tensor_copy(out=lo, in_=r2)
                    planes.append(lo)

            for r in range(n_row_tiles):
                pr = min(ROW_TILE, ROWS - r * ROW_TILE)
                ot = opool.tile([ROW_TILE, D], f32)
                oh_sl = oh[:, r * ROW_TILE : r * ROW_TILE + pr]
                if mode == "f32r":
                    oh_sl = oh_sl.bitcast(mybir.dt.float32r)
                for c in range(n_col_tiles):
                    cn = min(COL_TILE, D - c * COL_TILE)
                    ps = ppool.tile([ROW_TILE, COL_TILE], f32)
                    for pi, plane in enumerate(planes):
                        rhs = plane[:, c * COL_TILE : c * COL_TILE + cn]
                        if mode == "f32r":
                            rhs = rhs.bitcast(mybir.dt.float32r)
                        nc.tensor.matmul(
                            ps[:pr, :cn],
                            oh_sl,
                            rhs,
                            start=(pi == 0),
                            stop=(pi == len(planes) - 1),
                        )
                    nc.vector.tensor_copy(
                        out=ot[:pr, c * COL_TILE : c * COL_TILE + cn],
                        in_=ps[:pr, :cn],
                    )
                nc.sync.dma_start(
                    out=out[r * ROW_TILE : r * ROW_TILE + pr, :], in_=ot[:pr, :]
                )
    _install_multiwait_splitter(nc)
    return nc


_NC_CACHE: dict[str, bass.Bass] = {}


def _get_nc(mode: str) -> bass.Bass:
    if mode not in _NC_CACHE:
        _NC_CACHE[mode] = build_nc(mode)
    return _NC_CACHE[mode]


def run(inputs, labels, prototypes, mode: str = _MODE, **spmd_kwargs):
    """Run the kernel; returns (output, BassKernelResults)."""
    lbl = np.asarray(labels).reshape(B, L)
    proto = np.ascontiguousarray(
        np.asarray(prototypes, dtype=np.float32).reshape(N_PROTO, D)
    )
    if mode == "v2":
        import ml_dtypes

        table_input = {"planes": host_split_planes(proto)}
        lbl_dt = ml_dtypes.bfloat16
    elif mode == "v3":
        table_input = {"planes": host_split_planes2(proto)}
        lbl_dt = np.float32
    elif mode in ("v4", "v5", "v7", "v8"):
        import ml_dtypes

        pl = host_split_planes(proto)
        table_input = {
            "planesA": np.ascontiguousarray(pl[:, 0:1024]),
            "planesB": np.ascontiguousarray(pl[:, 1024:]),
        }
        lbl_dt = ml_dtypes.bfloat16
    elif mode == "v6":
        import ml_dtypes

        pl = host_split_planes_f8(proto)
        table_input = {
            "planesA": np.ascontiguousarray(pl[:, 0:1024]),
            "planesB": np.ascontiguousarray(pl[:, 1024:]),
            "cmpscl": host_cmpscl(),
        }
        lbl_dt = ml_dtypes.bfloat16
    else:
        table_input = {"proto": proto}
        lbl_dt = np.float32
    in_maps = []
    for c in range(N_CORES):
        lf = (
            lbl[c * B_PER_CORE : (c + 1) * B_PER_CORE]
            .reshape(1, ROWS)
            .astype(lbl_dt)
        )
        in_maps.append({"lbl": lf, **table_input})
    res = run_bass_kernel_spmd(
        _get_nc(mode), in_maps, core_ids=list(range(N_CORES)), **spmd_kwargs
    )
    outs = [
        r["out"].reshape(B_PER_CORE, L, NCHAN, T, F) for r in res.results
    ]
    return np.concatenate(outs, axis=0), res


def kernel(inputs, labels, prototypes):
    out, _ = run(inputs, labels, prototypes)
    return out



# revision 29
# speedup vs baseline: 3.4447x; 3.4447x over previous
"""Embedding-lookup kernel for Trainium2 (Bass/Tile), 8-core data-parallel.

Problem: out[b, l] = prototypes[labels[b, l]]
  inputs     (512, 21, 1, 29, 129) f32  -- unused except for batch size
  labels     (512, 21) int64            -- values in [0, 25)
  prototypes (25, 1, 29, 129) f32
  out        (512, 21, 1, 29, 129) f32  (~161 MB)

Strategy (memory regime): shard the batch dim across 8 cores (64 batches =
1344 lookups per core). Per core, keep the tiny prototype table resident in
SBUF, build a one-hot matrix from the labels on device (PE ones-matmul
broadcast + iota/is_equal), and perform the gather as one-hot @ table
matmuls on the PE, streaming PSUM -> SBUF -> DRAM. HBM traffic is then
write-only (20.1 MB per core), which is the roofline for this problem.

f32 exactness: the PE's fp32 matmul is a 2-pass approximation, so the table
is split into three bf16 planes (hi/mid/lo) whose sum reconstructs every f32
exactly. The planes are stacked along the contraction dim (partition groups
0/32/64, K=96) so a single bf16 matmul accumulates hi+mid+lo in fp32 PSUM;
with 0/1 one-hot weights the gathered values are bit-exact.

Measured on 8 axon trn2 cores: ~67-80 us HW exec (bit-exact), vs a ~56 us
per-core HBM write floor + ~16 us fixed framework preamble/teardown.
"""

import json

import numpy as np

import concourse.bass as bass
import concourse.mybir as mybir
from concourse.tile import TileContext
from concourse.bass_utils import run_bass_kernel_spmd

B, L, NCHAN, T, F = 512, 21, 1, 29, 129
D = NCHAN * T * F            # 3741 features per prototype
N_PROTO = 25
N_CORES = 8
B_PER_CORE = B // N_CORES    # 64
ROWS = B_PER_CORE * L        # 1344 lookups per core

ROW_TILE = 128               # output rows per matmul (PSUM partition dim)
COL_TILE = 512               # output cols per matmul (one PSUM bank of f32)

# "v2" (exact; host-split bf16 planes, one matmul per tile), "k75" (exact,
# fully on-device split), "bf16x3" (exact, three matmuls per tile),
# "f32"/"f32r" (native fp32 PE paths, speed/exactness probes only),
# "v3" (~1e-5 rel err; K=64 two-plane split, latency-optimized pipeline).
_MODE = "v2"

GP = 32                  # partition stride between the three plane groups
KDIM = 3 * GP            # 96 = matmul contraction dim incl. zero pads


def _split_multiwaits(bir: dict) -> int:
    """This walrus build allows at most one sync-wait per instruction on
    several instruction encodings; Tile attaches one wait per dependency.
    Hoist every wait of a multi-wait instruction into its own EventSemaphore
    (the encoding `wait_ge` uses) inserted directly before it on the same
    engine. Returns the number of instructions split."""
    n_split = 0
    ctr = 0
    for f in bir["functions"]:
        for blk in f["blocks"]:
            insts = blk["instructions"]
            out = []
            for inst in insts:
                si = inst.get("sync_info")
                waits = (si or {}).get("on_wait") or []
                if len(waits) > 1:
                    n_split += 1
                    for w in waits:
                        ctr += 1
                        out.append(
                            {
                                "debug": inst.get("debug", 0),
                                "engine": inst["engine"],
                                "ins": [],
                                "outs": [],
                                "name": f"mwsplit-{ctr}",
                                "opcode": "EventSemaphore",
                                "sync_info": {"on_update": [], "on_wait": [w]},
                            }
                        )
                    si["on_wait"] = []
                out.append(inst)
            blk["instructions"] = out
    return n_split


def _install_multiwait_splitter(nc: bass.Bass) -> None:
    orig = nc.to_json_bytes

    def patched() -> bytes:
        bir = json.loads(orig())
        _split_multiwaits(bir)
        return json.dumps(bir).encode()

    nc.to_json_bytes = patched


def host_split_planes(proto: np.ndarray) -> np.ndarray:
    """Split the f32 table into hi/mid/lo bf16 planes (sum reconstructs every
    f32 exactly) laid out at partitions 0/32/64 with zero pads."""
    import ml_dtypes

    bf = ml_dtypes.bfloat16
    x = proto.astype(np.float32).reshape(N_PROTO, D)
    hi = x.astype(bf)
    r1 = x - hi.astype(np.float32)
    mid = r1.astype(bf)
    r2 = r1 - mid.astype(np.float32)
    lo = r2.astype(bf)
    planes = np.zeros((KDIM, D), dtype=bf)
    planes[0:N_PROTO] = hi
    planes[GP : GP + N_PROTO] = mid
    planes[2 * GP : 2 * GP + N_PROTO] = lo
    return planes


def host_split_planes2(proto: np.ndarray) -> np.ndarray:
    """Two bf16 planes (hi/mid) at partitions 0/32; sum reconstructs f32 to
    ~2^-17 relative error (harness gate is 2e-2)."""
    import ml_dtypes

    bf = ml_dtypes.bfloat16
    x = proto.astype(np.float32).reshape(N_PROTO, D)
    hi = x.astype(bf)
    mid = (x - hi.astype(np.float32)).astype(bf)
    planes = np.zeros((2 * GP, D), dtype=bf)
    planes[0:N_PROTO] = hi
    planes[GP : GP + N_PROTO] = mid
    return planes


def build_nc_v3() -> bass.Bass:
    """Latency-optimized gather: K=64 (hi/mid bf16 planes at partitions 0/32),
    labels broadcast to 64 partitions by DMA (PE starts directly on gather
    matmuls), planes loaded in two chunks on the Activation HWDGE queue,
    first output tile written per column-pair, output DMAs alternating
    between the Sync and Activation queues."""
    f32 = mybir.dt.float32
    bf16 = mybir.dt.bfloat16
    i32 = mybir.dt.int32
    K2 = 2 * GP              # 64

    nc = bass.Bass()
    lbl = nc.dram_tensor("lbl", [1, ROWS], f32, kind="ExternalInput")
    planes_in = nc.dram_tensor("planes", [K2, D], bf16, kind="ExternalInput")
    out = nc.dram_tensor("out", [ROWS, D], f32, kind="ExternalOutput")

    n_row_tiles = (ROWS + ROW_TILE - 1) // ROW_TILE   # 11
    OH_CHUNK = 448
    PAIR = 2 * COL_TILE                               # 1024
    n_pairs = (D + PAIR - 1) // PAIR                  # 4

    with TileContext(nc) as tc:
        with (
            tc.tile_pool(name="const", bufs=1) as cpool,
            tc.tile_pool(name="psum", bufs=4, space="PSUM") as ppool,
            tc.tile_pool(name="outp", bufs=8) as opool,
        ):
            # --- inputs: label broadcast on Sync queue, planes on Act queue
            lbl75 = cpool.tile([K2, ROWS], f32)
            nc.sync.dma_start(
                out=lbl75, in_=lbl[0].partition_broadcast(K2)
            )
            planes = cpool.tile([K2, D], bf16)
            nc.scalar.dma_start(out=planes[:, 0:PAIR], in_=planes_in[:, 0:PAIR])
            nc.scalar.dma_start(out=planes[:, PAIR:], in_=planes_in[:, PAIR:])

            # --- per-partition compare value: p & 31, clamped to 25 on pads
            iota_i = cpool.tile([K2, 1], i32)
            nc.gpsimd.iota(iota_i, pattern=[[0, 1]], base=0, channel_multiplier=1)
            iota_q = cpool.tile([K2, 1], i32)
            nc.vector.tensor_scalar(
                out=iota_q, in0=iota_i, scalar1=GP - 1, scalar2=None,
                op0=mybir.AluOpType.bitwise_and,
            )
            iota_m = cpool.tile([K2, 1], i32)
            nc.vector.tensor_scalar(
                out=iota_m, in0=iota_q, scalar1=N_PROTO, scalar2=None,
                op0=mybir.AluOpType.min,
            )
            iota_f = cpool.tile([K2, 1], f32)
            nc.vector.tensor_copy(out=iota_f, in_=iota_m)

            oh = cpool.tile([K2, ROWS], bf16)

            def build_oh_chunk(ch: int) -> None:
                c0 = ch * OH_CHUNK
                cw = min(OH_CHUNK, ROWS - c0)
                nc.vector.tensor_scalar(
                    out=oh[:, c0 : c0 + cw],
                    in0=lbl75[:, c0 : c0 + cw],
                    scalar1=iota_f[:, 0:1],
                    scalar2=None,
                    op0=mybir.AluOpType.is_equal,
                )

            build_oh_chunk(0)

            for r in range(n_row_tiles):
                pr = min(ROW_TILE, ROWS - r * ROW_TILE)
                ot = opool.tile([ROW_TILE, D], f32)
                oh_sl = oh[:, r * ROW_TILE : r * ROW_TILE + pr]
                for p in range(n_pairs):
                    c0 = p * PAIR
                    cw = min(PAIR, D - c0)
                    ps = ppool.tile([ROW_TILE, PAIR], f32)
                    for h in range(2):
                        hw = min(COL_TILE, cw - h * COL_TILE)
                        if hw <= 0:
                            break
                        nc.tensor.matmul(
                            ps[:pr, h * COL_TILE : h * COL_TILE + hw],
                            oh_sl,
                            planes[:, c0 + h * COL_TILE : c0 + h * COL_TILE + hw],
                            start=True,
                            stop=True,
                        )
                    dst = ot[:pr, c0 : c0 + cw]
                    if p % 2 == 0:
                        nc.vector.tensor_copy(out=dst, in_=ps[:pr, :cw])
                    else:
                        nc.scalar.copy(out=dst, in_=ps[:pr, :cw])
                    if r == 0:
                        # stream tile 0 per pair, alternating queues
                        eng = nc.sync if p % 2 == 0 else nc.scalar
                        eng.dma_start(
                            out=out[0:pr, c0 : c0 + cw],
                            in_=ot[:pr, c0 : c0 + cw],
                        )
                    elif r == 1 and p == 1:
                        nc.sync.dma_start(
                            out=out[ROW_TILE : ROW_TILE + pr, 0 : 2 * PAIR],
                            in_=ot[:pr, 0 : 2 * PAIR],
                        )
                if r == 1:
                    nc.scalar.dma_start(
                        out=out[ROW_TILE : ROW_TILE + pr, 2 * PAIR :],
                        in_=ot[:pr, 2 * PAIR :],
                    )
                elif r > 1:
                    eng = nc.sync if r % 2 == 0 else nc.scalar
                    eng.dma_start(
                        out=out[r * ROW_TILE : r * ROW_TILE + pr, :],
                        in_=ot[:pr, :],
                    )
                if r == 0:
                    build_oh_chunk(1)
                elif r == 1:
                    build_oh_chunk(2)
    _install_multiwait_splitter(nc)
    return nc


def build_nc_v4() -> bass.Bass:
    """v2's exact K=96 gather with a latency-optimized pipeline:
    - planes staged as two contiguous DRAM tensors, loaded on the Act queue
      (fully contiguous packets at full engine rate, off the Sync queue),
    - one-hot built per 448-row chunk, interleaved with the first tiles,
    - tile 0 streamed out per column-pair, tile 1 in halves,
    - output DMAs alternate between the Sync and Act HWDGE queues."""
    f32 = mybir.dt.float32
    bf16 = mybir.dt.bfloat16
    i32 = mybir.dt.int32
    PAIR = 2 * COL_TILE                               # 1024
    DA = 1024                                         # planesA cols
    n_pairs = (D + PAIR - 1) // PAIR                  # 4

    nc = bass.Bass()
    lbl = nc.dram_tensor("lbl", [1, ROWS], bf16, kind="ExternalInput")
    planesA_in = nc.dram_tensor("planesA", [KDIM, DA], bf16, kind="ExternalInput")
    planesB_in = nc.dram_tensor(
        "planesB", [KDIM, D - DA], bf16, kind="ExternalInput"
    )
    out = nc.dram_tensor("out", [ROWS, D], f32, kind="ExternalOutput")

    n_row_tiles = (ROWS + ROW_TILE - 1) // ROW_TILE   # 11
    OH_CHUNK = 448

    with TileContext(nc) as tc:
        with (
            tc.tile_pool(name="const", bufs=1) as cpool,
            tc.tile_pool(name="psum", bufs=4, space="PSUM") as ppool,
            tc.tile_pool(name="outp", bufs=8) as opool,
        ):
            lblsb = cpool.tile([1, ROWS], bf16)
            nc.sync.dma_start(out=lblsb, in_=lbl[:])

            planes = cpool.tile([KDIM, D], bf16)
            nc.scalar.dma_start(out=planes[:, 0:DA], in_=planesA_in[:])
            nc.scalar.dma_start(out=planes[:, DA:], in_=planesB_in[:])

            ones = cpool.tile([1, KDIM], bf16)
            nc.vector.memset(ones, 1.0)
            iota_i = cpool.tile([KDIM, 1], i32)
            nc.gpsimd.iota(iota_i, pattern=[[0, 1]], base=0, channel_multiplier=1)
            iota_q = cpool.tile([KDIM, 1], i32)
            nc.vector.tensor_scalar(
                out=iota_q, in0=iota_i, scalar1=GP - 1, scalar2=None,
                op0=mybir.AluOpType.bitwise_and,
            )
            iota_m = cpool.tile([KDIM, 1], i32)
            nc.vector.tensor_scalar(
                out=iota_m, in0=iota_q, scalar1=N_PROTO, scalar2=None,
                op0=mybir.AluOpType.min,
            )
            iota_f = cpool.tile([KDIM, 1], f32)
            nc.vector.tensor_copy(out=iota_f, in_=iota_m)

            oh = cpool.tile([KDIM, ROWS], bf16)

            def build_oh_chunk(ch: int) -> None:
                c0 = ch * OH_CHUNK
                cw = min(OH_CHUNK, ROWS - c0)
                pb = ppool.tile([ROW_TILE, PAIR], f32, tag="ps")
                nc.tensor.matmul(
                    pb[:KDIM, :cw],
                    ones[0:1, :],
                    lblsb[0:1, c0 : c0 + cw],
                    start=True,
                    stop=True,
                )
                nc.vector.tensor_scalar(
                    out=oh[:, c0 : c0 + cw],
                    in0=pb[:KDIM, :cw],
                    scalar1=iota_f[:, 0:1],
                    scalar2=None,
                    op0=mybir.AluOpType.is_equal,
                )

            build_oh_chunk(0)

            for r in range(n_row_tiles):
                pr = min(ROW_TILE, ROWS - r * ROW_TILE)
                ot = opool.tile([ROW_TILE, D], f32)
                oh_sl = oh[:, r * ROW_TILE : r * ROW_TILE + pr]
                for p in range(n_pairs):
                    c0 = p * PAIR
                    cw = min(PAIR, D - c0)
                    ps = ppool.tile([ROW_TILE, PAIR], f32, tag="ps")
                    for h in range(2):
                        hw = min(COL_TILE, cw - h * COL_TILE)
                        if hw <= 0:
                            break
                        nc.tensor.matmul(
                            ps[:pr, h * COL_TILE : h * COL_TILE + hw],
                            oh_sl,
                            planes[:, c0 + h * COL_TILE : c0 + h * COL_TILE + hw],
                            start=True,
                            stop=True,
                        )
                    dst = ot[:pr, c0 : c0 + cw]
                    if p % 2 == 0:
                        nc.vector.tensor_copy(out=dst, in_=ps[:pr, :cw])
                    else:
                        nc.scalar.copy(out=dst, in_=ps[:pr, :cw])
                    if r == 0:
                        eng = nc.sync if p % 2 == 0 else nc.scalar
                        eng.dma_start(
                            out=out[0:pr, c0 : c0 + cw],
                            in_=ot[:pr, c0 : c0 + cw],
                        )
                    elif r == 1 and p == 1:
                        nc.sync.dma_start(
                            out=out[ROW_TILE : ROW_TILE + pr, 0 : 2 * PAIR],
                            in_=ot[:pr, 0 : 2 * PAIR],
                        )
                if r == 1:
                    nc.scalar.dma_start(
                        out=out[ROW_TILE : ROW_TILE + pr, 2 * PAIR :],
                        in_=ot[:pr, 2 * PAIR :],
                    )
                elif r > 1:
                    eng = nc.sync if r % 2 == 0 else nc.scalar
                    eng.dma_start(
                        out=out[r * ROW_TILE : r * ROW_TILE + pr, :],
                        in_=ot[:pr, :],
                    )
                if r == 0:
                    build_oh_chunk(1)
                elif r == 1:
                    build_oh_chunk(2)
    _install_multiwait_splitter(nc)
    return nc


def build_nc_v5() -> bass.Bass:
    """Grant-aware overlap: epoch-1 HBM stays light (sync-queue only, tile-0
    pair DMAs ~1.9MB) so the power governor grants full PE clock at ~20us;
    after that the stream goes wide: per-half-tile DMAs split between the
    Sync and Act HWDGE queues, so each DMA waits on only two PSUM copies."""
    f32 = mybir.dt.float32
    bf16 = mybir.dt.bfloat16
    i32 = mybir.dt.int32
    PAIR = 2 * COL_TILE                               # 1024
    DA = 1024                                         # planesA cols
    n_pairs = (D + PAIR - 1) // PAIR                  # 4
    HALF = 2 * PAIR                                   # 2048 cols per DMA half

    nc = bass.Bass()
    lbl = nc.dram_tensor("lbl", [1, ROWS], bf16, kind="ExternalInput")
    planesA_in = nc.dram_tensor("planesA", [KDIM, DA], bf16, kind="ExternalInput")
    planesB_in = nc.dram_tensor(
        "planesB", [KDIM, D - DA], bf16, kind="ExternalInput"
    )
    out = nc.dram_tensor("out", [ROWS, D], f32, kind="ExternalOutput")

    n_row_tiles = (ROWS + ROW_TILE - 1) // ROW_TILE   # 11
    OH_CHUNK = 448

    with TileContext(nc) as tc:
        with (
            tc.tile_pool(name="const", bufs=1) as cpool,
            tc.tile_pool(name="psum", bufs=4, space="PSUM") as ppool,
            tc.tile_pool(name="outp", bufs=8) as opool,
        ):
            lblsb = cpool.tile([1, ROWS], bf16)
            nc.sync.dma_start(out=lblsb, in_=lbl[:])

            planes = cpool.tile([KDIM, D], bf16)
            nc.sync.dma_start(out=planes[:, 0:DA], in_=planesA_in[:])
            nc.sync.dma_start(out=planes[:, DA:], in_=planesB_in[:])

            ones = cpool.tile([1, KDIM], bf16)
            nc.vector.memset(ones, 1.0)
            iota_i = cpool.tile([KDIM, 1], i32)
            nc.gpsimd.iota(iota_i, pattern=[[0, 1]], base=0, channel_multiplier=1)
            iota_q = cpool.tile([KDIM, 1], i32)
            nc.vector.tensor_scalar(
                out=iota_q, in0=iota_i, scalar1=GP - 1, scalar2=None,
                op0=mybir.AluOpType.bitwise_and,
            )
            iota_m = cpool.tile([KDIM, 1], i32)
            nc.vector.tensor_scalar(
                out=iota_m, in0=iota_q, scalar1=N_PROTO, scalar2=None,
                op0=mybir.AluOpType.min,
            )
            iota_f = cpool.tile([KDIM, 1], f32)
            nc.vector.tensor_copy(out=iota_f, in_=iota_m)

            oh = cpool.tile([KDIM, ROWS], bf16)

            def build_oh_chunk(ch: int) -> None:
                c0 = ch * OH_CHUNK
                cw = min(OH_CHUNK, ROWS - c0)
                pb = ppool.tile([ROW_TILE, PAIR], f32, tag="ps")
                nc.tensor.matmul(
                    pb[:KDIM, :cw],
                    ones[0:1, :],
                    lblsb[0:1, c0 : c0 + cw],
                    start=True,
                    stop=True,
                )
                nc.vector.tensor_scalar(
                    out=oh[:, c0 : c0 + cw],
                    in0=pb[:KDIM, :cw],
                    scalar1=iota_f[:, 0:1],
                    scalar2=None,
                    op0=mybir.AluOpType.is_equal,
                )

            build_oh_chunk(0)

            for r in range(n_row_tiles):
                pr = min(ROW_TILE, ROWS - r * ROW_TILE)
                r0 = r * ROW_TILE
                ot = opool.tile([ROW_TILE, D], f32)
                oh_sl = oh[:, r0 : r0 + pr]
                for p in range(n_pairs):
                    c0 = p * PAIR
                    cw = min(PAIR, D - c0)
                    ps = ppool.tile([ROW_TILE, PAIR], f32, tag="ps")
                    for h in range(2):
                        hw = min(COL_TILE, cw - h * COL_TILE)
                        if hw <= 0:
                            break
                        nc.tensor.matmul(
                            ps[:pr, h * COL_TILE : h * COL_TILE + hw],
                            oh_sl,
                            planes[:, c0 + h * COL_TILE : c0 + h * COL_TILE + hw],
                            start=True,
                            stop=True,
                        )
                    dst = ot[:pr, c0 : c0 + cw]
                    if p % 2 == 0:
                        nc.vector.tensor_copy(out=dst, in_=ps[:pr, :cw])
                    else:
                        nc.scalar.copy(out=dst, in_=ps[:pr, :cw])
                    if r == 0:
                        # epoch-1: keep traffic on the sync queue only
                        nc.sync.dma_start(
                            out=out[0:pr, c0 : c0 + cw],
                            in_=ot[:pr, c0 : c0 + cw],
                        )
                    elif p == 1:
                        # first half (cols 0:2048) after its two copies
                        nc.sync.dma_start(
                            out=out[r0 : r0 + pr, 0:HALF], in_=ot[:pr, 0:HALF]
                        )
                if r > 0:
                    nc.scalar.dma_start(
                        out=out[r0 : r0 + pr, HALF:], in_=ot[:pr, HALF:]
                    )
                if r == 0:
                    build_oh_chunk(1)
                elif r == 1:
                    build_oh_chunk(2)
    _install_multiwait_splitter(nc)
    return nc


K4 = 128                     # v6 contraction dim (5 packed fp8 plane groups)
N_PLANES = 5                 # planes packed at stride N_PROTO (25)
F8_SHIFT = 4                 # per-plane residual scale 2^4 (e4m3 mantissa)


def host_split_planes_f8(proto: np.ndarray) -> np.ndarray:
    """Five fp8-e4m3 planes packed at partition stride 25 (partitions
    0-124). Residuals are scaled by 2^4 per level (the matching one-hot
    carries 2^-4g in bf16), reconstructing f32 to ~2^-20 relative error with
    an absolute floor of 2^-26 — safe for the ~2^-16 minimum prototype
    magnitude. fp8 moving data runs the PE at 2x bf16 throughput."""
    import ml_dtypes

    f8 = ml_dtypes.float8_e4m3
    s = float(2 ** F8_SHIFT)
    x = proto.astype(np.float32).reshape(N_PROTO, D)
    planes = np.zeros((K4, D), dtype=f8)
    r = x
    for g in range(N_PLANES):
        p = r.astype(f8)
        planes[g * N_PROTO : (g + 1) * N_PROTO] = p
        r = (r - p.astype(np.float32)) * s
    return planes


def host_cmpscl() -> np.ndarray:
    """[128, 2] f32: col 0 = per-partition compare value (p mod 25, sentinel
    25 on pad partitions), col 1 = one-hot magnitude 2^(-4 * group)."""
    p = np.arange(K4)
    g = p // N_PROTO
    cmp = np.where(p < N_PLANES * N_PROTO, p % N_PROTO, N_PROTO).astype(np.float32)
    scl = np.where(
        p < N_PLANES * N_PROTO, 2.0 ** (-F8_SHIFT * g), 0.0
    ).astype(np.float32)
    return np.stack([cmp, scl], axis=1)


def build_nc_v6() -> bass.Bass:
    """fp8 gather with v2's proven DMA pattern: all output DMAs on the Sync
    queue, tile 0 primed per column pair, tiles 1+ with one DMA per tile and
    two PSUM->SBUF copies ([128,2048] DVE + [128,1693] ACT). The fp8 moving
    operand keeps the PE producer ahead of the drain regardless of the
    clock-governor state, so the stream is DMA-limited end to end."""
    f32 = mybir.dt.float32
    bf16 = mybir.dt.bfloat16
    f8 = mybir.dt.float8e4
    PAIR = 2 * COL_TILE                               # 1024
    HALF = 2 * PAIR                                   # 2048
    DA = 1024

    nc = bass.Bass()
    lbl = nc.dram_tensor("lbl", [1, ROWS], bf16, kind="ExternalInput")
    cs_in = nc.dram_tensor("cmpscl", [K4, 2], f32, kind="ExternalInput")
    planesA_in = nc.dram_tensor("planesA", [K4, DA], f8, kind="ExternalInput")
    planesB_in = nc.dram_tensor("planesB", [K4, D - DA], f8, kind="ExternalInput")
    out = nc.dram_tensor("out", [ROWS, D], f32, kind="ExternalOutput")

    n_row_tiles = (ROWS + ROW_TILE - 1) // ROW_TILE   # 11
    OH_CHUNK = 448

    with TileContext(nc) as tc:
        with (
            tc.tile_pool(name="const", bufs=1) as cpool,
            tc.tile_pool(name="psum", bufs=4, space="PSUM") as ppool,
            tc.tile_pool(name="outp", bufs=8) as opool,
        ):
            lblsb = cpool.tile([1, ROWS], bf16)
            nc.sync.dma_start(out=lblsb, in_=lbl[:])
            cs = cpool.tile([K4, 2], f32)
            nc.sync.dma_start(out=cs, in_=cs_in[:])
            planes = cpool.tile([K4, D], f8)
            nc.sync.dma_start(out=planes[:, 0:DA], in_=planesA_in[:])
            nc.sync.dma_start(out=planes[:, DA:], in_=planesB_in[:])

            ones = cpool.tile([1, K4], bf16)
            nc.vector.memset(ones, 1.0)

            oh = cpool.tile([K4, ROWS], bf16)

            def build_oh_chunk(ch: int) -> None:
                c0 = ch * OH_CHUNK
                cw = min(OH_CHUNK, ROWS - c0)
                pb = ppool.tile([ROW_TILE, PAIR], f32, tag="ps")
                nc.tensor.matmul(
                    pb[:K4, :cw],
                    ones[0:1, :],
                    lblsb[0:1, c0 : c0 + cw],
                    start=True,
                    stop=True,
                )
                nc.vector.tensor_scalar(
                    out=oh[:, c0 : c0 + cw],
                    in0=pb[:K4, :cw],
                    scalar1=cs[:, 0:1],
                    scalar2=cs[:, 1:2],
                    op0=mybir.AluOpType.is_equal,
                    op1=mybir.AluOpType.mult,
                )

            build_oh_chunk(0)

            for r in range(n_row_tiles):
                pr = min(ROW_TILE, ROWS - r * ROW_TILE)
                r0 = r * ROW_TILE
                ot = opool.tile([ROW_TILE, D], f32)
                oh_sl = oh[:, r0 : r0 + pr]
                for p in range(4):
                    c0 = p * PAIR
                    cw = min(PAIR, D - c0)
                    ps = ppool.tile([ROW_TILE, PAIR], f32, tag="ps")
                    for h in range(2):
                        hw = min(COL_TILE, cw - h * COL_TILE)
                        if hw <= 0:
                            break
                        nc.tensor.matmul(
                            ps[:pr, h * COL_TILE : h * COL_TILE + hw],
                            oh_sl,
                            planes[:, c0 + h * COL_TILE : c0 + h * COL_TILE + hw],
                            start=True,
                            stop=True,
                        )
                    dst = ot[:pr, c0 : c0 + cw]
                    if p % 2 == 0:
                        nc.vector.tensor_copy(out=dst, in_=ps[:pr, :cw])
                    else:
                        nc.scalar.copy(out=dst, in_=ps[:pr, :cw])
                    if r == 0:
                        # prime per column pair for the earliest first write
                        nc.sync.dma_start(
                            out=out[0:pr, c0 : c0 + cw],
                            in_=ot[:pr, c0 : c0 + cw],
                        )
                if r > 0:
                    nc.sync.dma_start(
                        out=out[r0 : r0 + pr, :], in_=ot[:pr, :]
                    )
                if r == 0:
                    build_oh_chunk(1)
                elif r == 1:
                    build_oh_chunk(2)
    _install_multiwait_splitter(nc)
    return nc


def build_nc_v7() -> bass.Bass:
    """v2 with two low-risk deltas: planes staged as two contiguous DRAM
    tensors (2 sync DMAs instead of 8 strided chunk loads -> sync engine free
    ~2.5us earlier, so the one-hot build and tile-0 primes start sooner) and
    a deeper output pool (10 bufs) so the producer never waits on slots."""
    f32 = mybir.dt.float32
    bf16 = mybir.dt.bfloat16
    i32 = mybir.dt.int32
    DA = 1024

    nc = bass.Bass()
    lbl = nc.dram_tensor("lbl", [1, ROWS], bf16, kind="ExternalInput")
    planesA_in = nc.dram_tensor("planesA", [KDIM, DA], bf16, kind="ExternalInput")
    planesB_in = nc.dram_tensor(
        "planesB", [KDIM, D - DA], bf16, kind="ExternalInput"
    )
    out = nc.dram_tensor("out", [ROWS, D], f32, kind="ExternalOutput")

    n_row_tiles = (ROWS + ROW_TILE - 1) // ROW_TILE
    n_col_tiles = (D + COL_TILE - 1) // COL_TILE
    OH_CHUNK = 448
    n_oh_chunks = (ROWS + OH_CHUNK - 1) // OH_CHUNK

    with TileContext(nc) as tc:
        with (
            tc.tile_pool(name="const", bufs=1) as cpool,
            tc.tile_pool(name="psum", bufs=4, space="PSUM") as ppool,
            tc.tile_pool(name="outp", bufs=10) as opool,
        ):
            lblsb = cpool.tile([1, ROWS], bf16)
            nc.sync.dma_start(out=lblsb, in_=lbl[:])

            planes = cpool.tile([KDIM, D], bf16)
            nc.sync.dma_start(out=planes[:, 0:DA], in_=planesA_in[:])
            nc.sync.dma_start(out=planes[:, DA:], in_=planesB_in[:])

            ones = cpool.tile([1, KDIM], bf16)
            nc.vector.memset(ones, 1.0)

            iota_i = cpool.tile([KDIM, 1], i32)
            nc.gpsimd.iota(iota_i, pattern=[[0, 1]], base=0, channel_multiplier=1)
            iota_q = cpool.tile([KDIM, 1], i32)
            nc.vector.tensor_scalar(
                out=iota_q, in0=iota_i, scalar1=GP - 1, scalar2=None,
                op0=mybir.AluOpType.bitwise_and,
            )
            iota_m = cpool.tile([KDIM, 1], i32)
            nc.vector.tensor_scalar(
                out=iota_m, in0=iota_q, scalar1=N_PROTO, scalar2=None,
                op0=mybir.AluOpType.min,
            )
            iota_f = cpool.tile([KDIM, 1], f32)
            nc.vector.tensor_copy(out=iota_f, in_=iota_m)

            oh = cpool.tile([KDIM, ROWS], bf16)
            for ch in range(n_oh_chunks):
                cw = min(OH_CHUNK, ROWS - ch * OH_CHUNK)
                pb = ppool.tile([ROW_TILE, COL_TILE], f32, tag="ps")
                nc.tensor.matmul(
                    pb[:KDIM, :cw],
                    ones[0:1, :],
                    lblsb[0:1, ch * OH_CHUNK : ch * OH_CHUNK + cw],
                    start=True,
                    stop=True,
                )
                nc.vector.tensor_scalar(
                    out=oh[:, ch * OH_CHUNK : ch * OH_CHUNK + cw],
                    in0=pb[:KDIM, :cw],
                    scalar1=iota_f[:, 0:1],
                    scalar2=None,
                    op0=mybir.AluOpType.is_equal,
                )

            n_pairs = (n_col_tiles + 1) // 2
            for r in range(n_row_tiles):
                pr = min(ROW_TILE, ROWS - r * ROW_TILE)
                ot = opool.tile([ROW_TILE, D], f32)
                oh_sl = oh[:, r * ROW_TILE : r * ROW_TILE + pr]
                for cp in range(n_pairs):
                    c0 = 2 * cp * COL_TILE
                    cw = min(2 * COL_TILE, D - c0)
                    ps = ppool.tile([ROW_TILE, 2 * COL_TILE], f32, tag="ps")
                    for h in range(2):
                        hw = min(COL_TILE, cw - h * COL_TILE)
                        if hw <= 0:
                            break
                        nc.tensor.matmul(
                            ps[:pr, h * COL_TILE : h * COL_TILE + hw],
                            oh_sl,
                            planes[:, c0 + h * COL_TILE : c0 + h * COL_TILE + hw],
                            start=True,
                            stop=True,
                        )
                    dst = ot[:pr, c0 : c0 + cw]
                    if cp % 2 == 1:
                        nc.scalar.copy(out=dst, in_=ps[:pr, :cw])
                    else:
                        nc.vector.tensor_copy(out=dst, in_=ps[:pr, :cw])
                    if r == 0 and cp in (0, 1):
                        nc.sync.dma_start(
                            out=out[0:pr, c0 : c0 + cw],
                            in_=ot[:pr, c0 : c0 + cw],
                        )
                if r == 0:
                    nc.sync.dma_start(
                        out=out[0:pr, 4 * COL_TILE :],
                        in_=ot[:pr, 4 * COL_TILE :],
                    )
                else:
                    nc.sync.dma_start(
                        out=out[r * ROW_TILE : r * ROW_TILE + pr, :], in_=ot[:pr, :]
                    )
    _install_multiwait_splitter(nc)
    return nc


def build_nc_v8() -> bass.Bass:
    """v7 with per-tile DMA row rotation: each tile's output DMA is split at
    a varying row offset so the row->DMA-engine assignment rotates tile to
    tile, spreading address-linked slow patches (HBM contention bursts that
    otherwise pile onto one engine) across all 16 engines."""
    f32 = mybir.dt.float32
    bf16 = mybir.dt.bfloat16
    i32 = mybir.dt.int32
    DA = 1024

    nc = bass.Bass()
    lbl = nc.dram_tensor("lbl", [1, ROWS], bf16, kind="ExternalInput")
    planesA_in = nc.dram_tensor("planesA", [KDIM, DA], bf16, kind="ExternalInput")
    planesB_in = nc.dram_tensor(
        "planesB", [KDIM, D - DA], bf16, kind="ExternalInput"
    )
    out = nc.dram_tensor("out", [ROWS, D], f32, kind="ExternalOutput")

    n_row_tiles = (ROWS + ROW_TILE - 1) // ROW_TILE
    n_col_tiles = (D + COL_TILE - 1) // COL_TILE
    OH_CHUNK = 448
    n_oh_chunks = (ROWS + OH_CHUNK - 1) // OH_CHUNK

    with TileContext(nc) as tc:
        with (
            tc.tile_pool(name="const", bufs=1) as cpool,
            tc.tile_pool(name="psum", bufs=4, space="PSUM") as ppool,
            tc.tile_pool(name="outp", bufs=8) as opool,
        ):
            lblsb = cpool.tile([1, ROWS], bf16)
            nc.sync.dma_start(out=lblsb, in_=lbl[:])

            planes = cpool.tile([KDIM, D], bf16)
            nc.sync.dma_start(out=planes[:, 0:DA], in_=planesA_in[:])
            nc.sync.dma_start(out=planes[:, DA:], in_=planesB_in[:])

            ones = cpool.tile([1, KDIM], bf16)
            nc.vector.memset(ones, 1.0)

            iota_i = cpool.tile([KDIM, 1], i32)
            nc.gpsimd.iota(iota_i, pattern=[[0, 1]], base=0, channel_multiplier=1)
            iota_q = cpool.tile([KDIM, 1], i32)
            nc.vector.tensor_scalar(
                out=iota_q, in0=iota_i, scalar1=GP - 1, scalar2=None,
                op0=mybir.AluOpType.bitwise_and,
            )
            iota_m = cpool.tile([KDIM, 1], i32)
            nc.vector.tensor_scalar(
                out=iota_m, in0=iota_q, scalar1=N_PROTO, scalar2=None,
                op0=mybir.AluOpType.min,
            )
            iota_f = cpool.tile([KDIM, 1], f32)
            nc.vector.tensor_copy(out=iota_f, in_=iota_m)

            oh = cpool.tile([KDIM, ROWS], bf16)
            for ch in range(n_oh_chunks):
                cw = min(OH_CHUNK, ROWS - ch * OH_CHUNK)
                pb = ppool.tile([ROW_TILE, COL_TILE], f32, tag="ps")
                nc.tensor.matmul(
                    pb[:KDIM, :cw],
                    ones[0:1, :],
                    lblsb[0:1, ch * OH_CHUNK : ch * OH_CHUNK + cw],
                    start=True,
                    stop=True,
                )
                nc.vector.tensor_scalar(
                    out=oh[:, ch * OH_CHUNK : ch * OH_CHUNK + cw],
                    in0=pb[:KDIM, :cw],
                    scalar1=iota_f[:, 0:1],
                    scalar2=None,
                    op0=mybir.AluOpType.is_equal,
                )

            n_pairs = (n_col_tiles + 1) // 2
            for r in range(n_row_tiles):
                pr = min(ROW_TILE, ROWS - r * ROW_TILE)
                r0 = r * ROW_TILE
                ot = opool.tile([ROW_TILE, D], f32)
                oh_sl = oh[:, r0 : r0 + pr]
                for cp in range(n_pairs):
                    c0 = 2 * cp * COL_TILE
                    cw = min(2 * COL_TILE, D - c0)
                    ps = ppool.tile([ROW_TILE, 2 * COL_TILE], f32, tag="ps")
                    for h in range(2):
                        hw = min(COL_TILE, cw - h * COL_TILE)
                        if hw <= 0:
                            break
                        nc.tensor.matmul(
                            ps[:pr, h * COL_TILE : h * COL_TILE + hw],
                            oh_sl,
                            planes[:, c0 + h * COL_TILE : c0 + h * COL_TILE + hw],
                            start=True,
                            stop=True,
                        )
                    dst = ot[:pr, c0 : c0 + cw]
                    if cp % 2 == 1:
                        nc.scalar.copy(out=dst, in_=ps[:pr, :cw])
                    else:
                        nc.vector.tensor_copy(out=dst, in_=ps[:pr, :cw])
                    if r == 0 and cp in (0, 1):
                        nc.sync.dma_start(
                            out=out[0:pr, c0 : c0 + cw],
                            in_=ot[:pr, c0 : c0 + cw],
                        )
                if r == 0:
                    nc.sync.dma_start(
                        out=out[0:pr, 4 * COL_TILE :],
                        in_=ot[:pr, 4 * COL_TILE :],
                    )
                else:
                    s = (5 * r) % 16
                    if s == 0 or pr < ROW_TILE:
                        nc.sync.dma_start(
                            out=out[r0 : r0 + pr, :], in_=ot[:pr, :]
                        )
                    else:
                        nc.sync.dma_start(
                            out=out[r0 + s : r0 + pr, :], in_=ot[s:pr, :]
                        )
                        nc.sync.dma_start(
                            out=out[r0 : r0 + s, :], in_=ot[:s, :]
                        )
    _install_multiwait_splitter(nc)
    return nc


def build_nc_v9() -> bass.Bass:
    """v2 with planes staged as two contiguous DRAM tensors loaded on the
    Act HWDGE queue: the sync queue carries only labels + output writes (its
    descriptor pattern stays v2-like), while the planes land in parallel."""
    f32 = mybir.dt.float32
    bf16 = mybir.dt.bfloat16
    i32 = mybir.dt.int32
    DA = 1024

    nc = bass.Bass()
    lbl = nc.dram_tensor("lbl", [1, ROWS], bf16, kind="ExternalInput")
    planesA_in = nc.dram_tensor("planesA", [KDIM, DA], bf16, kind="ExternalInput")
    planesB_in = nc.dram_tensor(
        "planesB", [KDIM, D - DA], bf16, kind="ExternalInput"
    )
    out = nc.dram_tensor("out", [ROWS, D], f32, kind="ExternalOutput")

    n_row_tiles = (ROWS + ROW_TILE - 1) // ROW_TILE
    n_col_tiles = (D + COL_TILE - 1) // COL_TILE
    OH_CHUNK = 448
    n_oh_chunks = (ROWS + OH_CHUNK - 1) // OH_CHUNK

    with TileContext(nc) as tc:
        with (
            tc.tile_pool(name="const", bufs=1) as cpool,
            tc.tile_pool(name="psum", bufs=4, space="PSUM") as ppool,
            tc.tile_pool(name="outp", bufs=8) as opool,
        ):
            lblsb = cpool.tile([1, ROWS], bf16)
            nc.sync.dma_start(out=lblsb, in_=lbl[:])

            planes = cpool.tile([KDIM, D], bf16)
            nc.scalar.dma_start(out=planes[:, 0:DA], in_=planesA_in[:])
            nc.scalar.dma_start(out=planes[:, DA:], in_=planesB_in[:])

            ones = cpool.tile([1, KDIM], bf16)
            nc.vector.memset(ones, 1.0)

            iota_i = cpool.tile([KDIM, 1], i32)
            nc.gpsimd.iota(iota_i, pattern=[[0, 1]], base=0, channel_multiplier=1)
            iota_q = cpool.tile([KDIM, 1], i32)
            nc.vector.tensor_scalar(
                out=iota_q, in0=iota_i, scalar1=GP - 1, scalar2=None,
                op0=mybir.AluOpType.bitwise_and,
            )
            iota_m = cpool.tile([KDIM, 1], i32)
            nc.vector.tensor_scalar(
                out=iota_m, in0=iota_q, scalar1=N_PROTO, scalar2=None,
                op0=mybir.AluOpType.min,
            )
            iota_f = cpool.tile([KDIM, 1], f32)
            nc.vector.tensor_copy(out=iota_f, in_=iota_m)

            oh = cpool.tile([KDIM, ROWS], bf16)
            for ch in range(n_oh_chunks):
                cw = min(OH_CHUNK, ROWS - ch * OH_CHUNK)
                pb = ppool.tile([ROW_TILE, COL_TILE], f32, tag="ps")
                nc.tensor.matmul(
                    pb[:KDIM, :cw],
                    ones[0:1, :],
                    lblsb[0:1, ch * OH_CHUNK : ch * OH_CHUNK + cw],
                    start=True,
                    stop=True,
                )
                nc.vector.tensor_scalar(
                    out=oh[:, ch * OH_CHUNK : ch * OH_CHUNK + cw],
                    in0=pb[:KDIM, :cw],
                    scalar1=iota_f[:, 0:1],
                    scalar2=None,
                    op0=mybir.AluOpType.is_equal,
                )

            n_pairs = (n_col_tiles + 1) // 2
            for r in range(n_row_tiles):
                pr = min(ROW_TILE, ROWS - r * ROW_TILE)
                ot = opool.tile([ROW_TILE, D], f32)
                oh_sl = oh[:, r * ROW_TILE : r * ROW_TILE + pr]
                for cp in range(n_pairs):
                    c0 = 2 * cp * COL_TILE
                    cw = min(2 * COL_TILE, D - c0)
                    ps = ppool.tile([ROW_TILE, 2 * COL_TILE], f32, tag="ps")
                    for h in range(2):
                        hw = min(COL_TILE, cw - h * COL_TILE)
                        if hw <= 0:
                            break
                        nc.tensor.matmul(
                            ps[:pr, h * COL_TILE : h * COL_TILE + hw],
                            oh_sl,
                            planes[:, c0 + h * COL_TILE : c0 + h * COL_TILE + hw],
                            start=True,
                            stop=True,
                        )
                    dst = ot[:pr, c0 : c0 + cw]
                    if cp % 2 == 1:
                        nc.scalar.copy(out=dst, in_=ps[:pr, :cw])
                    else:
                        nc.vector.tensor_copy(out=dst, in_=ps[:pr, :cw])
                    if r == 0 and cp in (0, 1):
                        nc.sync.dma_start(
                            out=out[0:pr, c0 : c0 + cw],
                            in_=ot[:pr, c0 : c0 + cw],
                        )
                if r == 0:
                    nc.sync.dma_start(
                        out=out[0:pr, 4 * COL_TILE :],
                        in_=ot[:pr, 4 * COL_TILE :],
                    )
                else:
                    nc.sync.dma_start(
                        out=out[r * ROW_TILE : r * ROW_TILE + pr, :], in_=ot[:pr, :]
                    )
    _install_multiwait_splitter(nc)
    return nc


def build_nc_v8() -> bass.Bass:
    """v7 with per-tile DMA row rotation: each tile's output DMA is split at
    a varying row offset so the row->DMA-engine assignment rotates tile to
    tile, spreading address-linked slow patches (HBM contention bursts that
    otherwise pile onto one engine) across all 16 engines."""
    f32 = mybir.dt.float32
    bf16 = mybir.dt.bfloat16
    i32 = mybir.dt.int32
    DA = 1024

    nc = bass.Bass()
    lbl = nc.dram_tensor("lbl", [1, ROWS], bf16, kind="ExternalInput")
    planesA_in = nc.dram_tensor("planesA", [KDIM, DA], bf16, kind="ExternalInput")
    planesB_in = nc.dram_tensor(
        "planesB", [KDIM, D - DA], bf16, kind="ExternalInput"
    )
    out = nc.dram_tensor("out", [ROWS, D], f32, kind="ExternalOutput")

    n_row_tiles = (ROWS + ROW_TILE - 1) // ROW_TILE
    n_col_tiles = (D + COL_TILE - 1) // COL_TILE
    OH_CHUNK = 448
    n_oh_chunks = (ROWS + OH_CHUNK - 1) // OH_CHUNK

    with TileContext(nc) as tc:
        with (
            tc.tile_pool(name="const", bufs=1) as cpool,
            tc.tile_pool(name="psum", bufs=4, space="PSUM") as ppool,
            tc.tile_pool(name="outp", bufs=8) as opool,
        ):
            lblsb = cpool.tile([1, ROWS], bf16)
            nc.sync.dma_start(out=lblsb, in_=lbl[:])

            planes = cpool.tile([KDIM, D], bf16)
            nc.scalar.dma_start(out=planes[:, 0:DA], in_=planesA_in[:])
            nc.scalar.dma_start(out=planes[:, DA:], in_=planesB_in[:])

            ones = cpool.tile([1, KDIM], bf16)
            nc.vector.memset(ones, 1.0)

            iota_i = cpool.tile([KDIM, 1], i32)
            nc.gpsimd.iota(iota_i, pattern=[[0, 1]], base=0, channel_multiplier=1)
            iota_q = cpool.tile([KDIM, 1], i32)
            nc.vector.tensor_scalar(
                out=iota_q, in0=iota_i, scalar1=GP - 1, scalar2=None,
                op0=mybir.AluOpType.bitwise_and,
            )
            iota_m = cpool.tile([KDIM, 1], i32)
            nc.vector.tensor_scalar(
                out=iota_m, in0=iota_q, scalar1=N_PROTO, scalar2=None,
                op0=mybir.AluOpType.min,
            )
            iota_f = cpool.tile([KDIM, 1], f32)
            nc.vector.tensor_copy(out=iota_f, in_=iota_m)

            oh = cpool.tile([KDIM, ROWS], bf16)
            for ch in range(n_oh_chunks):
                cw = min(OH_CHUNK, ROWS - ch * OH_CHUNK)
                pb = ppool.tile([ROW_TILE, COL_TILE], f32, tag="ps")
                nc.tensor.matmul(
                    pb[:KDIM, :cw],
                    ones[0:1, :],
                    lblsb[0:1, ch * OH_CHUNK : ch * OH_CHUNK + cw],
                    start=True,
                    stop=True,
                )
                nc.vector.tensor_scalar(
                    out=oh[:, ch * OH_CHUNK : ch * OH_CHUNK + cw],
                    in0=pb[:KDIM, :cw],
                    scalar1=iota_f[:, 0:1],
                    scalar2=None,
                    op0=mybir.AluOpType.is_equal,
                )

            n_pairs = (n_col_tiles + 1) // 2
            for r in range(n_row_tiles):
                pr = min(ROW_TILE, ROWS - r * ROW_TILE)
                r0 = r * ROW_TILE
                ot = opool.tile([ROW_TILE, D], f32)
                oh_sl = oh[:, r0 : r0 + pr]
                for cp in range(n_pairs):
                    c0 = 2 * cp * COL_TILE
                    cw = min(2 * COL_TILE, D - c0)
                    ps = ppool.tile([ROW_TILE, 2 * COL_TILE], f32, tag="ps")
                    for h in range(2):
                        hw = min(COL_TILE, cw - h * COL_TILE)
                        if hw <= 0:
                            break
                        nc.tensor.matmul(
                            ps[:pr, h * COL_TILE : h * COL_TILE + hw],
                            oh_sl,
                            planes[:, c0 + h * COL_TILE : c0 + h * COL_TILE + hw],
                            start=True,
                            stop=True,
                        )
                    dst = ot[:pr, c0 : c0 + cw]
                    if cp % 2 == 1:
                        nc.scalar.copy(out=dst, in_=ps[:pr, :cw])
                    else:
                        nc.vector.tensor_copy(out=dst, in_=ps[:pr, :cw])
                    if r == 0 and cp in (0, 1):
                        nc.sync.dma_start(
                            out=out[0:pr, c0 : c0 + cw],
                            in_=ot[:pr, c0 : c0 + cw],
                        )
                if r == 0:
                    nc.sync.dma_start(
                        out=out[0:pr, 4 * COL_TILE :],
                        in_=ot[:pr, 4 * COL_TILE :],
                    )
                else:
                    s = (5 * r) % 16
                    if s == 0 or pr < ROW_TILE:
                        nc.sync.dma_start(
                            out=out[r0 : r0 + pr, :], in_=ot[:pr, :]
                        )
                    else:
                        nc.sync.dma_start(
                            out=out[r0 + s : r0 + pr, :], in_=ot[s:pr, :]
                        )
                        nc.sync.dma_start(
                            out=out[r0 : r0 + s, :], in_=ot[:s, :]
                        )
    _install_multiwait_splitter(nc)
    return nc


def build_nc_v2() -> bass.Bass:
    """Gather as one-hot @ planes matmul, K=96 (three bf16 planes of the
    table stacked along the contraction dim, pre-split on host). One matmul
    per 128x512 output tile; PSUM->SBUF copies alternate DVE/ACT; one DMA
    per 128-row tile."""
    f32 = mybir.dt.float32
    bf16 = mybir.dt.bfloat16
    i32 = mybir.dt.int32

    nc = bass.Bass()
    lbl = nc.dram_tensor("lbl", [1, ROWS], bf16, kind="ExternalInput")
    planes_in = nc.dram_tensor("planes", [KDIM, D], bf16, kind="ExternalInput")
    out = nc.dram_tensor("out", [ROWS, D], f32, kind="ExternalOutput")

    n_row_tiles = (ROWS + ROW_TILE - 1) // ROW_TILE
    n_col_tiles = (D + COL_TILE - 1) // COL_TILE
    OH_CHUNK = 448
    n_oh_chunks = (ROWS + OH_CHUNK - 1) // OH_CHUNK

    with TileContext(nc) as tc:
        with (
            tc.tile_pool(name="const", bufs=1) as cpool,
            tc.tile_pool(name="psum", bufs=4, space="PSUM") as ppool,
            tc.tile_pool(name="outp", bufs=8) as opool,
        ):
            lblsb = cpool.tile([1, ROWS], bf16)
            nc.sync.dma_start(out=lblsb, in_=lbl[:])

            planes = cpool.tile([KDIM, D], bf16)
            for c in range(n_col_tiles):
                cn = min(COL_TILE, D - c * COL_TILE)
                nc.sync.dma_start(
                    out=planes[:, c * COL_TILE : c * COL_TILE + cn],
                    in_=planes_in[:, c * COL_TILE : c * COL_TILE + cn],
                )
            ones = cpool.tile([1, KDIM], bf16)
            nc.vector.memset(ones, 1.0)

            iota_i = cpool.tile([KDIM, 1], i32)
            nc.gpsimd.iota(iota_i, pattern=[[0, 1]], base=0, channel_multiplier=1)
            iota_q = cpool.tile([KDIM, 1], i32)
            nc.vector.tensor_scalar(
                out=iota_q, in0=iota_i, scalar1=GP - 1, scalar2=None,
                op0=mybir.AluOpType.bitwise_and,
            )
            iota_m = cpool.tile([KDIM, 1], i32)
            nc.vector.tensor_scalar(
                out=iota_m, in0=iota_q, scalar1=N_PROTO, scalar2=None,
                op0=mybir.AluOpType.min,
            )
            iota_f = cpool.tile([KDIM, 1], f32)
            nc.vector.tensor_copy(out=iota_f, in_=iota_m)

            # broadcast labels to 96 partitions on the (idle) PE: ones^T @ lbl,
            # then compare against the per-partition group-local iota
            oh = cpool.tile([KDIM, ROWS], bf16)
            for ch in range(n_oh_chunks):
                cw = min(OH_CHUNK, ROWS - ch * OH_CHUNK)
                pb = ppool.tile([ROW_TILE, COL_TILE], f32, tag="ps")
                nc.tensor.matmul(
                    pb[:KDIM, :cw],
                    ones[0:1, :],
                    lblsb[0:1, ch * OH_CHUNK : ch * OH_CHUNK + cw],
                    start=True,
                    stop=True,
                )
                nc.vector.tensor_scalar(
                    out=oh[:, ch * OH_CHUNK : ch * OH_CHUNK + cw],
                    in0=pb[:KDIM, :cw],
                    scalar1=iota_f[:, 0:1],
                    scalar2=None,
                    op0=mybir.AluOpType.is_equal,
                )

            n_pairs = (n_col_tiles + 1) // 2
            for r in range(n_row_tiles):
                pr = min(ROW_TILE, ROWS - r * ROW_TILE)
                ot = opool.tile([ROW_TILE, D], f32)
                oh_sl = oh[:, r * ROW_TILE : r * ROW_TILE + pr]
                for cp in range(n_pairs):
                    c0 = 2 * cp * COL_TILE
                    cw = min(2 * COL_TILE, D - c0)
                    ps = ppool.tile([ROW_TILE, 2 * COL_TILE], f32)
                    for h in range(2):
                        hw = min(COL_TILE, cw - h * COL_TILE)
                        if hw <= 0:
                            break
                        nc.tensor.matmul(
                            ps[:pr, h * COL_TILE : h * COL_TILE + hw],
                            oh_sl,
                            planes[:, c0 + h * COL_TILE : c0 + h * COL_TILE + hw],
                            start=True,
                            stop=True,
                        )
                    dst = ot[:pr, c0 : c0 + cw]
                    if cp % 2 == 1:
                        nc.scalar.copy(out=dst, in_=ps[:pr, :cw])
                    else:
                        nc.vector.tensor_copy(out=dst, in_=ps[:pr, :cw])
                    if r == 0 and cp in (0, 1):
                        # prime the output-DMA stream before the tile finishes
                        nc.sync.dma_start(
                            out=out[0:pr, c0 : c0 + cw],
                            in_=ot[:pr, c0 : c0 + cw],
                        )
                if r == 0:
                    nc.sync.dma_start(
                        out=out[0:pr, 4 * COL_TILE :],
                        in_=ot[:pr, 4 * COL_TILE :],
                    )
                else:
                    nc.sync.dma_start(
                        out=out[r * ROW_TILE : r * ROW_TILE + pr, :], in_=ot[:pr, :]
                    )
    _install_multiwait_splitter(nc)
    return nc


def build_nc_k75() -> bass.Bass:
    """One matmul per output tile: stationary is the 25-row one-hot stacked
    three times along the contraction dim, the moving operand is the
    hi/mid/lo bf16 table planes stacked the same way. PSUM accumulates
    hi+mid+lo in fp32 in a single pass -> bit-exact f32 gather.

    Compute-engine SBUF accesses must start at a 32-aligned partition, so the
    three 25-row groups sit at partitions 0/32/64 (K=96). Pad partitions:
    one-hot rows compare labels against 25 (never matches -> 0), plane pad
    rows are zeroed via DMA so 0*0 keeps PSUM clean."""
    f32 = mybir.dt.float32
    bf16 = mybir.dt.bfloat16
    i32 = mybir.dt.int32
    GP = 32                  # partition stride between plane groups
    P3 = 3 * GP              # 96 = contraction dim incl. pads

    nc = bass.Bass()
    lbl = nc.dram_tensor("lbl", [1, ROWS], f32, kind="ExternalInput")
    proto = nc.dram_tensor("proto", [N_PROTO, D], f32, kind="ExternalInput")
    out = nc.dram_tensor("out", [ROWS, D], f32, kind="ExternalOutput")

    n_row_tiles = (ROWS + ROW_TILE - 1) // ROW_TILE
    n_col_tiles = (D + COL_TILE - 1) // COL_TILE

    with TileContext(nc) as tc:
        with (
            tc.tile_pool(name="const", bufs=1) as cpool,
            tc.tile_pool(name="psum", bufs=8, space="PSUM") as ppool,
            tc.tile_pool(name="outp", bufs=4) as opool,
        ):
            tbl75 = cpool.tile([P3, D], f32)
            lbl75 = cpool.tile([P3, ROWS], f32)
            for g in range(3):
                sl = slice(g * GP, g * GP + N_PROTO)
                nc.sync.dma_start(out=tbl75[sl, :], in_=proto[:])
                nc.sync.dma_start(
                    out=lbl75[g * GP : (g + 1) * GP, :],
                    in_=lbl[0].partition_broadcast(GP),
                )

            iota_i = cpool.tile([P3, 1], i32)
            nc.gpsimd.iota(iota_i, pattern=[[0, 1]], base=0, channel_multiplier=1)
            # group-local index, pads clamp to 25 which no label ever equals
            iota_q = cpool.tile([P3, 1], i32)
            nc.vector.tensor_scalar(
                out=iota_q, in0=iota_i, scalar1=GP - 1, scalar2=None,
                op0=mybir.AluOpType.bitwise_and,
            )
            iota_m = cpool.tile([P3, 1], i32)
            nc.vector.tensor_scalar(
                out=iota_m, in0=iota_q, scalar1=N_PROTO, scalar2=None,
                op0=mybir.AluOpType.min,
            )
            iota_f = cpool.tile([P3, 1], f32)
            nc.vector.tensor_copy(out=iota_f, in_=iota_m)

            oh = cpool.tile([P3, ROWS], bf16)
            nc.vector.tensor_scalar(
                out=oh, in0=lbl75, scalar1=iota_f[:, 0:1], scalar2=None,
                op0=mybir.AluOpType.is_equal,
            )

            # planes: partitions 0-24 hi, 32-56 mid, 64-88 lo (bf16, RN)
            planes = cpool.tile([P3, D], bf16)
            scrA = cpool.tile([P3, D], f32)
            scrB = cpool.tile([P3, D], f32)
            zpad = cpool.tile([GP - N_PROTO, D], bf16)
            nc.vector.memset(zpad, 0.0)
            for g in range(3):
                nc.sync.dma_start(
                    out=planes[g * GP + N_PROTO : (g + 1) * GP, :], in_=zpad
                )
            s0 = slice(0, N_PROTO)
            s1 = slice(GP, GP + N_PROTO)
            s2 = slice(2 * GP, 2 * GP + N_PROTO)
            # hi plane
            nc.vector.tensor_copy(out=planes[s0, :], in_=tbl75[s0, :])
            # mid plane: cast(x - f32(bf16(x)))
            nc.vector.tensor_copy(out=planes[s1, :], in_=tbl75[s1, :])
            nc.vector.tensor_copy(out=scrA[s1, :], in_=planes[s1, :])
            nc.vector.tensor_sub(out=planes[s1, :], in0=tbl75[s1, :], in1=scrA[s1, :])
            # lo plane: r1 = x - hi_f; mid = bf16(r1); lo = bf16(r1 - f32(mid))
            nc.vector.tensor_copy(out=planes[s2, :], in_=tbl75[s2, :])
            nc.vector.tensor_copy(out=scrA[s2, :], in_=planes[s2, :])
            nc.vector.tensor_sub(out=scrB[s2, :], in0=tbl75[s2, :], in1=scrA[s2, :])
            nc.vector.tensor_copy(out=planes[s2, :], in_=scrB[s2, :])
            nc.vector.tensor_copy(out=scrA[s2, :], in_=planes[s2, :])
            nc.vector.tensor_sub(out=planes[s2, :], in0=scrB[s2, :], in1=scrA[s2, :])

            for r in range(n_row_tiles):
                pr = min(ROW_TILE, ROWS - r * ROW_TILE)
                ot = opool.tile([ROW_TILE, D], f32)
                oh_sl = oh[:, r * ROW_TILE : r * ROW_TILE + pr]
                for c in range(n_col_tiles):
                    cn = min(COL_TILE, D - c * COL_TILE)
                    ps = ppool.tile([ROW_TILE, COL_TILE], f32)
                    nc.tensor.matmul(
                        ps[:pr, :cn],
                        oh_sl,
                        planes[:, c * COL_TILE : c * COL_TILE + cn],
                        start=True,
                        stop=True,
                    )
                    dst = ot[:pr, c * COL_TILE : c * COL_TILE + cn]
                    if c in (3, 7):
                        nc.scalar.copy(out=dst, in_=ps[:pr, :cn])
                    else:
                        nc.vector.tensor_copy(out=dst, in_=ps[:pr, :cn])
                nc.sync.dma_start(
                    out=out[r * ROW_TILE : r * ROW_TILE + pr, :], in_=ot[:pr, :]
                )
    _install_multiwait_splitter(nc)
    return nc


def build_nc(mode: str = _MODE) -> bass.Bass:
    if mode == "v2":
        return build_nc_v2()
    if mode == "v3":
        return build_nc_v3()
    if mode == "v4":
        return build_nc_v4()
    if mode == "v5":
        return build_nc_v5()
    if mode == "v6":
        return build_nc_v6()
    if mode == "v7":
        return build_nc_v7()
    if mode == "v8":
        return build_nc_v8()
    if mode == "v9":
        return build_nc_v9()
    if mode == "k75":
        return build_nc_k75()
    f32 = mybir.dt.float32
    bf16 = mybir.dt.bfloat16

    nc = bass.Bass()
    lbl = nc.dram_tensor("lbl", [1, ROWS], f32, kind="ExternalInput")
    proto = nc.dram_tensor("proto", [N_PROTO, D], f32, kind="ExternalInput")
    out = nc.dram_tensor("out", [ROWS, D], f32, kind="ExternalOutput")

    n_row_tiles = (ROWS + ROW_TILE - 1) // ROW_TILE
    n_col_tiles = (D + COL_TILE - 1) // COL_TILE

    with TileContext(nc) as tc:
        with (
            tc.tile_pool(name="const", bufs=1) as cpool,
            tc.tile_pool(name="psum", bufs=8, space="PSUM") as ppool,
            tc.tile_pool(name="outp", bufs=4) as opool,
        ):
            tbl = cpool.tile([N_PROTO, D], f32)
            nc.sync.dma_start(out=tbl, in_=proto[:])

            lblb = cpool.tile([N_PROTO, ROWS], f32)
            nc.sync.dma_start(out=lblb, in_=lbl[0].partition_broadcast(N_PROTO))

            iot = cpool.tile([N_PROTO, 1], f32)
            nc.gpsimd.iota(
                iot,
                pattern=[[0, 1]],
                base=0,
                channel_multiplier=1,
                allow_small_or_imprecise_dtypes=True,
            )

            oh_dt = f32 if mode in ("f32", "f32r") else bf16
            oh = cpool.tile([N_PROTO, ROWS], oh_dt)
            nc.vector.tensor_scalar(
                out=oh,
                in0=lblb,
                scalar1=iot[:, 0:1],
                scalar2=None,
                op0=mybir.AluOpType.is_equal,
            )

            if mode in ("f32", "f32r"):
                planes = [tbl]
            else:
                # Exact f32 = hi + mid + lo, each bf16 (RN cast at each step).
                hi = cpool.tile([N_PROTO, D], bf16)
                nc.vector.tensor_copy(out=hi, in_=tbl)
                hi_f = cpool.tile([N_PROTO, D], f32)
                nc.vector.tensor_copy(out=hi_f, in_=hi)
                r1 = cpool.tile([N_PROTO, D], f32)
                nc.vector.tensor_sub(out=r1, in0=tbl, in1=hi_f)
                mid = cpool.tile([N_PROTO, D], bf16)
                nc.vector.tensor_copy(out=mid, in_=r1)
                planes = [hi, mid]
                if mode == "bf16x3":
                    mid_f = cpool.tile([N_PROTO, D], f32)
                    nc.vector.tensor_copy(out=mid_f, in_=mid)
                    r2 = cpool.tile([N_PROTO, D], f32)
                    nc.vector.tensor_sub(out=r2, in0=r1, in1=mid_f)
                    lo = cpool.tile([N_PROTO, D], bf16)
                    nc.vector.tensor_copy(out=lo, in_=r2)
                    planes.append(lo)

            for r in range(n_row_tiles):
                pr = min(ROW_TILE, ROWS - r * ROW_TILE)
                ot = opool.tile([ROW_TILE, D], f32)
                oh_sl = oh[:, r * ROW_TILE : r * ROW_TILE + pr]
                if mode == "f32r":
                    oh_sl = oh_sl.bitcast(mybir.dt.float32r)
                for c in range(n_col_tiles):
                    cn = min(COL_TILE, D - c * COL_TILE)
                    ps = ppool.tile([ROW_TILE, COL_TILE], f32)
                    for pi, plane in enumerate(planes):
                        rhs = plane[:, c * COL_TILE : c * COL_TILE + cn]
                        if mode == "f32r":
                            rhs = rhs.bitcast(mybir.dt.float32r)
                        nc.tensor.matmul(
                            ps[:pr, :cn],
                            oh_sl,
                            rhs,
                            start=(pi == 0),
                            stop=(pi == len(planes) - 1),
                        )
                    nc.vector.tensor_copy(
                        out=ot[:pr, c * COL_TILE : c * COL_TILE + cn],
                        in_=ps[:pr, :cn],
                    )
                nc.sync.dma_start(
                    out=out[r * ROW_TILE : r * ROW_TILE + pr, :], in_=ot[:pr, :]
                )
    _install_multiwait_splitter(nc)
    return nc


_NC_CACHE: dict[str, bass.Bass] = {}


def _get_nc(mode: str) -> bass.Bass:
    if mode not in _NC_CACHE:
        _NC_CACHE[mode] = build_nc(mode)
    return _NC_CACHE[mode]


def run(inputs, labels, prototypes, mode: str = _MODE, **spmd_kwargs):
    """Run the kernel; returns (output, BassKernelResults)."""
    lbl = np.asarray(labels).reshape(B, L)
    proto = np.ascontiguousarray(
        np.asarray(prototypes, dtype=np.float32).reshape(N_PROTO, D)
    )
    if mode == "v2":
        import ml_dtypes

        table_input = {"planes": host_split_planes(proto)}
        lbl_dt = ml_dtypes.bfloat16
    elif mode == "v3":
        table_input = {"planes": host_split_planes2(proto)}
        lbl_dt = np.float32
    elif mode in ("v4", "v5", "v7", "v8", "v9"):
        import ml_dtypes

        pl = host_split_planes(proto)
        table_input = {
            "planesA": np.ascontiguousarray(pl[:, 0:1024]),
            "planesB": np.ascontiguousarray(pl[:, 1024:]),
        }
        lbl_dt = ml_dtypes.bfloat16
    elif mode == "v6":
        import ml_dtypes

        pl = host_split_planes_f8(proto)
        table_input = {
            "planesA": np.ascontiguousarray(pl[:, 0:1024]),
            "planesB": np.ascontiguousarray(pl[:, 1024:]),
            "cmpscl": host_cmpscl(),
        }
        lbl_dt = ml_dtypes.bfloat16
    else:
        table_input = {"proto": proto}
        lbl_dt = np.float32
    in_maps = []
    for c in range(N_CORES):
        lf = (
            lbl[c * B_PER_CORE : (c + 1) * B_PER_CORE]
            .reshape(1, ROWS)
            .astype(lbl_dt)
        )
        in_maps.append({"lbl": lf, **table_input})
    res = run_bass_kernel_spmd(
        _get_nc(mode), in_maps, core_ids=list(range(N_CORES)), **spmd_kwargs
    )
    outs = [
        r["out"].reshape(B_PER_CORE, L, NCHAN, T, F) for r in res.results
    ]
    return np.concatenate(outs, axis=0), res


def kernel(inputs, labels, prototypes):
    out, _ = run(inputs, labels, prototypes)
    return out

